# revision 11
# baseline (speedup 1.0000x reference)
"""Channel-attention (XCA-style) Trainium2 kernel, 8-way SPMD — v2.

Shapes (hardcoded): B=4, N=16384, D=256, H=2 heads, c=128.
Sharding: core ci -> batch b=ci//2, token half ci%2 (T=8192 tokens/core).

Covariance formulation: per core, exactly LayerNorm the bf16 token tiles
(one fused scale+bias op per tile), accumulate three 256x256 token-
contracted Grams (M_SS, M_RS, M_RR) plus channel sums on the PE, then
  G    = Wq' M_SR Wk'^T + rank-1 bias outer-products   (head-diag blocks)
  dq/dk = diag(Wq' M_SS Wq'^T) + bias terms            (eye-dot on PE out)
One pair AllReduce of [128, 260] (G | dq | dk). Post-collective, softmax
gives attn; attn@v and the output projection collapse into a single
256x256 effective weight W_eff = Wo . blockdiag(attn_h) . Wv', applied to
the (transposed) normalized R in one matmul pass; per-token work in the
output phase is a single PSUM+bias-row evacuation. Output lands bf16 in
DRAM; the host upcasts to fp32.
"""
import sys, types

sys.path.insert(0, "/opt/trn_rl_repo")

try:
    import antenv
    if "antenv.axon_hooks" not in sys.modules:
        _hooks = types.ModuleType("antenv.axon_hooks")
        _hooks._hook = None
        _hooks.set_axon_ntff_profile_hook = lambda h: setattr(_hooks, "_hook", h)
        _hooks.get_axon_ntff_profile_hook = lambda: _hooks._hook
        sys.modules["antenv.axon_hooks"] = _hooks
        antenv.axon_hooks = _hooks
        from trn_agent_boot.trn_boot import _ntff_profile_via_ctypes
        _hooks.set_axon_ntff_profile_hook(
            _ntff_profile_via_ctypes("/opt/axon/libaxon_pjrt.so"))
except Exception:
    pass

import numpy as np
import ml_dtypes

import concourse.bass as bass
import concourse.bacc as bacc
import concourse.mybir as mybir
import concourse.tile as tile
from concourse.bass_utils import run_bass_kernel_spmd

BF16 = ml_dtypes.bfloat16
F32 = mybir.dt.float32
BF = mybir.dt.bfloat16
AL = mybir.AluOpType
AF = mybir.ActivationFunctionType
AX = mybir.AxisListType

B, N, D, H = 4, 16384, 256, 2
C = D // H
T = N // 2                 # tokens per core
NT = T // 128              # 64 token tiles / core
EPS_LN = 1e-5
EPS_NORM = 1e-12
N_CORES = 8
CHT = 4                    # token tiles per chunk
NCH = NT // CHT            # 8 chunks
PAYW = 260                 # collective payload width (G 256 | dq 2 | dk 2)

_nc_cache = {}


def _build_nc():
    nc = bacc.Bacc("TRN2", target_bir_lowering=False, debug=False,
                   num_devices=N_CORES)

    def ein(name, shape, dt=F32):
        return nc.dram_tensor(name, list(shape), dt, kind="ExternalInput")

    d_s = ein("x_s", [T, D])            # q source shard (input_S)
    d_r = ein("x_r", [T, D])            # kv source shard (input_R)
    d_wqT = ein("wqT", [D, D], BF)      # Wq'(=Wq.diag(lnS_w)) transposed [e,c]
    d_wkT = ein("wkT", [D, D], BF)
    d_wv = ein("wv", [D, D], BF)        # Wv' natural [c, e]
    d_woT = ein("woT", [D, D], BF)      # Wo transposed [c, o]
    d_rows = ein("rows", [1, 6 * D], BF)  # bq|bk|2bq|2bk|T*bq|T*bk rows
    d_bv = ein("bv_col", [128, H], BF)
    d_bo = ein("bo_col", [128, H])
    d_temp = ein("temp_col", [128, H])
    d_eyef = ein("eyef", [128, 128])
    d_eyeb = ein("eyeb", [128, 128], BF)
    d_out = nc.dram_tensor("out", [T, D], BF, kind="ExternalOutput")

    sv = d_s.rearrange("(j p) d -> p j d", p=128)
    rv = d_r.rearrange("(j p) d -> p j d", p=128)
    outv = d_out.rearrange("(j p) d -> p j d", p=128)

    with tile.TileContext(nc) as tc:
        import contextlib
        with contextlib.ExitStack() as ctx:
            _body(ctx, tc, nc, sv, rv, outv, d_wqT, d_wkT, d_wv, d_woT,
                  d_rows, d_bv, d_bo, d_temp, d_eyef, d_eyeb)
    nc.finalize()
    return nc


def _body(ctx, tc, nc, sv, rv, outv, d_wqT, d_wkT, d_wv, d_woT, d_rows,
          d_bv, d_bo, d_temp, d_eyef, d_eyeb):
    E = ctx.enter_context
    consts = E(tc.tile_pool(name="consts", bufs=1))
    stage = E(tc.tile_pool(name="stage", bufs=3))
    scrp = E(tc.tile_pool(name="scrp", bufs=2))
    nrm = E(tc.tile_pool(name="nrm", bufs=3))
    stp = E(tc.tile_pool(name="stp", bufs=3))
    pers = E(tc.tile_pool(name="pers", bufs=1))
    post = E(tc.tile_pool(name="post", bufs=1))
    small = E(tc.tile_pool(name="small", bufs=4))
    outp = E(tc.tile_pool(name="outp", bufs=2))
    dram = E(tc.tile_pool(name="dram", bufs=1, space="DRAM"))
    gacc = E(tc.tile_pool(name="gacc", bufs=1, space="PSUM"))

    # ---------------- constants ----------------
    wqT = consts.tile([128, 2, D], BF, tag="wqT")
    wkT = consts.tile([128, 2, D], BF, tag="wkT")
    wv_sb = consts.tile([128, 2, D], BF, tag="wv")
    woT = consts.tile([128, 2, D], BF, tag="woT")
    for dst, src in ((wqT, d_wqT), (wkT, d_wkT), (wv_sb, d_wv), (woT, d_woT)):
        nc.scalar.dma_start(out=dst[:], in_=src.rearrange("(h p) o -> p h o", p=128))
    rows_sb = consts.tile([1, 6 * D], BF, tag="rows")
    nc.scalar.dma_start(out=rows_sb[:], in_=d_rows[:, :])
    bq_row = rows_sb[0:1, 0 * D:1 * D]
    bk_row = rows_sb[0:1, 1 * D:2 * D]
    bq2_row = rows_sb[0:1, 2 * D:3 * D]
    bk2_row = rows_sb[0:1, 3 * D:4 * D]
    bqT_row = rows_sb[0:1, 4 * D:5 * D]
    bkT_row = rows_sb[0:1, 5 * D:6 * D]
    bv_col = consts.tile([128, H], BF, tag="bv")
    bo_col = consts.tile([128, H], F32, tag="bo")
    temp_col = consts.tile([128, H], F32, tag="temp")
    for dst, src in ((bv_col, d_bv), (bo_col, d_bo), (temp_col, d_temp)):
        nc.scalar.dma_start(out=dst[:], in_=src[:, :])
    eyef = consts.tile([128, 128], F32, tag="eyef")
    eyeb = consts.tile([128, 128], BF, tag="eyeb")
    nc.scalar.dma_start(out=eyef[:], in_=d_eyef[:, :])
    nc.scalar.dma_start(out=eyeb[:], in_=d_eyeb[:, :])
    ones_col = consts.tile([128, 1], BF, tag="ones_c")
    nc.vector.memset(ones_col[:], 1.0)
    ones_row = consts.tile([1, 128], BF, tag="ones_r")
    nc.gpsimd.memset(ones_row[:], 1.0)
    epsln = consts.tile([128, 1], F32, tag="epsln")
    nc.vector.memset(epsln[:], EPS_LN)
    zcol = consts.tile([128, 1], F32, tag="zcol")
    nc.vector.memset(zcol[:], 0.0)

    rdm = pers.tile([128, NT, 2, 128], BF, tag="rdm")   # normalized R, d-major

    psSS = gacc.tile([128, 2, 256], F32, tag="psSS")
    psRX = gacc.tile([128, 2, 512], F32, tag="psRX")    # [M_RS | M_RR] blocks
    psSum = gacc.tile([128, 512], F32, tag="psSum")     # row 0: [s_S | s_R]

    # ================= phase 1: stream chunks =================
    for ch in range(NCH):
        j0 = ch * CHT
        raw = stage.tile([128, 2, CHT, 256], BF, tag="raw")  # 0=S 1=R
        nc.gpsimd.dma_start(out=raw[:, 0], in_=sv[:, j0:j0 + CHT, :])
        nc.gpsimd.dma_start(out=raw[:, 1], in_=rv[:, j0:j0 + CHT, :])

        s1 = stp.tile([128, 2, CHT], F32, tag="s1")
        s2 = stp.tile([128, 2, CHT], F32, tag="s2")
        nc.vector.tensor_reduce(out=s1[:], in_=raw[:], axis=AX.X, op=AL.add)
        sq = scrp.tile([128, 2, CHT, 256], BF, tag="sq")
        nc.scalar.activation(out=sq[:], in_=raw[:], func=AF.Square,
                             bias=zcol[:, :], scale=1.0)
        nc.vector.tensor_reduce(out=s2[:], in_=sq[:], axis=AX.X, op=AL.add)

        mu = stp.tile([128, 2, CHT], F32, tag="mu")
        var = stp.tile([128, 2, CHT], F32, tag="var")
        sig = stp.tile([128, 2, CHT], F32, tag="sig")
        a_sc = stp.tile([128, 2, CHT], F32, tag="a_sc")
        b_sc = stp.tile([128, 2, CHT], F32, tag="b_sc")
        nc.vector.tensor_scalar(mu[:], s1[:], 1.0 / D, None, AL.mult)
        nc.vector.scalar_tensor_tensor(out=var[:], in0=mu[:], scalar=-1.0,
                                       op0=AL.mult, op1=AL.mult, in1=mu[:])
        nc.vector.scalar_tensor_tensor(out=var[:], in0=s2[:], scalar=1.0 / D,
                                       op0=AL.mult, op1=AL.add, in1=var[:])
        nc.scalar.activation(out=sig[:], in_=var[:], func=AF.Sqrt,
                             bias=epsln[:, :], scale=1.0)
        nc.vector.reciprocal(out=a_sc[:], in_=sig[:])
        nc.vector.scalar_tensor_tensor(out=b_sc[:], in0=mu[:], scalar=-1.0,
                                       op0=AL.mult, op1=AL.mult, in1=a_sc[:])

        nt = nrm.tile([128, 2, CHT, 256], BF, tag="nt")
        for jj in range(CHT):
            nc.vector.tensor_scalar(nt[:, 0, jj], raw[:, 0, jj],
                                    a_sc[:, 0, jj:jj + 1],
                                    b_sc[:, 0, jj:jj + 1], AL.mult, AL.add)
            nc.scalar.activation(out=nt[:, 1, jj], in_=raw[:, 1, jj],
                                 func=AF.Identity, bias=b_sc[:, 1, jj:jj + 1],
                                 scale=a_sc[:, 1, jj:jj + 1])

        for jj in range(CHT):
            j = j0 + jj
            st = (j == 0)
            sp = (j == NT - 1)
            for ech in range(2):
                nc.tensor.matmul(out=psSS[:, ech, :],
                                 lhsT=nt[:, 0, jj, ech * 128:(ech + 1) * 128],
                                 rhs=nt[:, 0, jj, :], start=st, stop=sp)
            for fch in range(2):
                nc.tensor.matmul(out=psRX[:, fch, :],
                                 lhsT=nt[:, 1, jj, fch * 128:(fch + 1) * 128],
                                 rhs=nt[:, :, jj, :], start=st, stop=sp)
            nc.tensor.matmul(out=psSum[0:1, :], lhsT=ones_col[:],
                             rhs=nt[:, :, jj, :], start=st, stop=sp)

        nc.sync.dma_start_transpose(rdm[:, j0:j0 + CHT, :, :], nt[:, 1])

    # ================= phase 2: local reductions -> payload ================
    mSS = post.tile([128, 2, 256], BF, tag="mSS")
    mRS = post.tile([128, 2, 256], BF, tag="mRS")
    mRR = post.tile([128, 2, 256], BF, tag="mRR")
    nc.vector.tensor_scalar(mSS[:], psSS[:], 1.0, None, AL.mult)
    nc.scalar.activation(out=mRS[:], in_=psRX[:, :, 0:256], func=AF.Identity,
                         bias=zcol[:, :], scale=1.0)
    nc.scalar.activation(out=mRR[:], in_=psRX[:, :, 256:512], func=AF.Identity,
                         bias=zcol[:, :], scale=1.0)
    sums_sb = post.tile([1, 512], F32, tag="sums")
    nc.vector.tensor_scalar(sums_sb[:], psSum[0:1, :], 1.0, None, AL.mult)

    with tc.tile_pool(name="pps", bufs=2, space="PSUM") as pps:
        # s_S / s_R rows -> bf16 columns via PE transpose
        ps_sc = pps.tile([128, 2, 512], F32, tag="ps")
        for k in range(4):
            nc.tensor.transpose(ps_sc[:, 0, k:k + 1],
                                sums_sb[0:1, k * 128:(k + 1) * 128],
                                eyef[0:1, 0:1])
        scol = post.tile([128, 4], BF, tag="scol")   # sS e0,e1 | sR e0,e1
        nc.vector.tensor_scalar(scol[:], ps_sc[:, 0, 0:4], 1.0, None, AL.mult)

        # (Wq' s_S) and (Wk' s_R) as rows [1, 256]
        ps_r = pps.tile([128, 2, 512], F32, tag="ps")
        for ech in range(2):
            nc.tensor.matmul(out=ps_r[0:1, 0, 0:256], lhsT=scol[:, ech:ech + 1],
                             rhs=wqT[:, ech, :], start=(ech == 0),
                             stop=(ech == 1))
            nc.tensor.matmul(out=ps_r[0:1, 0, 256:512],
                             lhsT=scol[:, 2 + ech:3 + ech],
                             rhs=wkT[:, ech, :], start=(ech == 0),
                             stop=(ech == 1))
        prows = post.tile([1, 512], BF, tag="prows")  # wqss row | wksr row
        nc.vector.tensor_scalar(prows[:], ps_r[0:1, 0, :], 1.0, None, AL.mult)
        wqss_row = prows[0:1, 0:256]
        wksr_row = prows[0:1, 256:512]

        # V = M_SR Wk'^T  (lhsT = M_RS blocks)
        ps_v = pps.tile([128, 2, 512], F32, tag="ps")
        for ech in range(2):
            for fch in range(2):
                nc.tensor.matmul(out=ps_v[:, ech, 0:256],
                                 lhsT=mRS[:, fch, ech * 128:(ech + 1) * 128],
                                 rhs=wkT[:, fch, :], start=(fch == 0),
                                 stop=(fch == 1))
        v_sb = post.tile([128, 2, 256], BF, tag="v_sb")
        nc.scalar.activation(out=v_sb[:], in_=ps_v[:, :, 0:256],
                             func=AF.Identity, bias=zcol[:, :], scale=1.0)

        # Z_q = M_SS Wq'^T-ish, Z_k = M_RR Wk'^T (for norm diagonals)
        ps_z = pps.tile([128, 2, 512], F32, tag="ps")
        for ech in range(2):
            for fch in range(2):
                nc.tensor.matmul(out=ps_z[:, ech, 0:256],
                                 lhsT=mSS[:, fch, ech * 128:(ech + 1) * 128],
                                 rhs=wqT[:, fch, :], start=(fch == 0),
                                 stop=(fch == 1))
                nc.tensor.matmul(out=ps_z[:, ech, 256:512],
                                 lhsT=mRR[:, fch, ech * 128:(ech + 1) * 128],
                                 rhs=wkT[:, fch, :], start=(fch == 0),
                                 stop=(fch == 1))
        z_sb = post.tile([128, 2, 512], BF, tag="z_sb")
        nc.vector.tensor_scalar(z_sb[:], ps_z[:], 1.0, None, AL.mult)

        payload = post.tile([128, PAYW], F32, tag="payload")

        # G head blocks + bias outers
        ps_g = pps.tile([128, 2, 512], F32, tag="ps")
        for hh in range(2):
            g = ps_g[:, 0, hh * 128:(hh + 1) * 128]
            hs = slice(hh * 128, (hh + 1) * 128)
            for ech in range(2):
                nc.tensor.matmul(out=g, lhsT=wqT[:, ech, hs],
                                 rhs=v_sb[:, ech, hs], start=(ech == 0),
                                 stop=False)
            nc.tensor.matmul(out=g, lhsT=bq_row[:, hs], rhs=wksr_row[:, hs],
                             start=False, stop=False)
            nc.tensor.matmul(out=g, lhsT=wqss_row[:, hs], rhs=bk_row[:, hs],
                             start=False, stop=False)
            nc.tensor.matmul(out=g, lhsT=bq_row[:, hs], rhs=bkT_row[:, hs],
                             start=False, stop=True)
        nc.vector.tensor_scalar(payload[:, 0:256], ps_g[:, 0, 0:256], 1.0,
                                None, AL.mult)

        # Y_q / Y_k head blocks + bias outers; eye-dot -> dq, dk
        ps_y = pps.tile([128, 2, 512], F32, tag="ps")
        for hh in range(2):
            hs = slice(hh * 128, (hh + 1) * 128)
            yq = ps_y[:, hh, 0:128]
            yk = ps_y[:, hh, 128:256]
            for ech in range(2):
                nc.tensor.matmul(out=yq, lhsT=wqT[:, ech, hs],
                                 rhs=z_sb[:, ech, hs], start=(ech == 0),
                                 stop=False)
            nc.tensor.matmul(out=yq, lhsT=bq2_row[:, hs], rhs=wqss_row[:, hs],
                             start=False, stop=False)
            nc.tensor.matmul(out=yq, lhsT=bq_row[:, hs], rhs=bqT_row[:, hs],
                             start=False, stop=True)
            for ech in range(2):
                nc.tensor.matmul(
                    out=yk, lhsT=wkT[:, ech, hs],
                    rhs=z_sb[:, ech, 256 + hh * 128:256 + (hh + 1) * 128],
                    start=(ech == 0), stop=False)
            nc.tensor.matmul(out=yk, lhsT=bk2_row[:, hs], rhs=wksr_row[:, hs],
                             start=False, stop=False)
            nc.tensor.matmul(out=yk, lhsT=bk_row[:, hs], rhs=bkT_row[:, hs],
                             start=False, stop=True)
            dscr = small.tile([128, 128], F32, tag="dscr")
            nc.vector.scalar_tensor_tensor(
                out=dscr[:], in0=yq, scalar=1.0, op0=AL.mult, op1=AL.mult,
                in1=eyef[:], accum_out=payload[:, 256 + hh:257 + hh])
            nc.vector.scalar_tensor_tensor(
                out=dscr[:], in0=yk, scalar=1.0, op0=AL.mult, op1=AL.mult,
                in1=eyef[:], accum_out=payload[:, 258 + hh:259 + hh])

        # ---------------- collective ----------------
        cc_in = dram.tile([128, PAYW], F32)
        cc_out = dram.tile([128, PAYW], F32)
        nc.gpsimd.dma_start(out=cc_in[:, :], in_=payload[:])
        nc.gpsimd.collective_compute(
            "AllReduce", AL.add,
            replica_groups=[[0, 1], [2, 3], [4, 5], [6, 7]],
            ins=[cc_in.opt()], outs=[cc_out.opt()])
        red = post.tile([128, PAYW], F32, tag="red")
        nc.sync.dma_start(out=red[:], in_=cc_out[:, :])

        # ---------------- phase 3: softmax + W_eff ----------------
        nrmc = small.tile([128, 4], F32, tag="nrmc")
        nc.scalar.activation(out=nrmc[:], in_=red[:, 256:260], func=AF.Sqrt,
                             bias=zcol[:, :], scale=1.0)
        nc.vector.tensor_scalar_max(nrmc[:], nrmc[:], EPS_NORM)
        nc.vector.reciprocal(out=nrmc[:], in_=nrmc[:])
        iq = small.tile([128, 2], F32, tag="iq")
        nc.vector.tensor_tensor(out=iq[:], in0=nrmc[:, 0:2], in1=temp_col[:],
                                op=AL.mult)

        # invk column -> broadcast tile via PE transpose + outer product
        ps_t = pps.tile([128, 2, 512], F32, tag="ps")
        ikrow = post.tile([1, 2, 128], BF, tag="ikrow")
        for hh in range(2):
            nc.tensor.transpose(ps_t[0:1, 0, hh * 128:(hh + 1) * 128],
                                nrmc[:, 2 + hh:3 + hh], eyef[:])
            nc.vector.tensor_scalar(
                ikrow[0:1, hh, :], ps_t[0:1, 0, hh * 128:(hh + 1) * 128],
                1.0, None, AL.mult)
        ps_ik = pps.tile([128, 2, 512], F32, tag="ps")
        for hh in range(2):
            nc.tensor.matmul(out=ps_ik[:, 0, hh * 128:(hh + 1) * 128],
                             lhsT=ones_row[:], rhs=ikrow[0:1, hh, :],
                             start=True, stop=True)
        ikb = post.tile([128, 2, 128], F32, tag="ikb")
        nc.vector.tensor_scalar(ikb[:], ps_ik[:, 0, 0:256], 1.0, None, AL.mult)

        # logits, softmax
        lg = post.tile([128, 2, 128], F32, tag="lg")
        for hh in range(2):
            nc.vector.tensor_scalar(lg[:, hh, :], red[:, hh * 128:(hh + 1) * 128],
                                    iq[:, hh:hh + 1], None, AL.mult)
        nc.vector.tensor_tensor(out=lg[:], in0=lg[:], in1=ikb[:], op=AL.mult)
        rmax = small.tile([128, 2], F32, tag="rmax")
        nc.vector.tensor_reduce(out=rmax[:], in_=lg[:], axis=AX.X, op=AL.max)
        nc.vector.tensor_scalar(rmax[:], rmax[:], -1.0, None, AL.mult)
        att = post.tile([128, 2, 128], F32, tag="att")
        for hh in range(2):
            nc.scalar.activation(out=att[:, hh, :], in_=lg[:, hh, :],
                                 func=AF.Exp, bias=rmax[:, hh:hh + 1],
                                 scale=1.0)
        rs = small.tile([128, 2], F32, tag="rs")
        nc.vector.tensor_reduce(out=rs[:], in_=att[:], axis=AX.X, op=AL.add)
        nc.vector.reciprocal(out=rs[:], in_=rs[:])
        attf = post.tile([128, 2, 128], F32, tag="attf")
        for hh in range(2):
            nc.vector.tensor_scalar(attf[:, hh, :], att[:, hh, :],
                                    rs[:, hh:hh + 1], None, AL.mult)

        # attn^T
        ps_at = pps.tile([128, 2, 512], F32, tag="ps")
        for hh in range(2):
            nc.tensor.transpose(ps_at[:, hh, 0:128], attf[:, hh, :], eyef[:])
        attT = post.tile([128, 2, 128], BF, tag="attT")
        nc.vector.tensor_scalar(attT[:], ps_at[:, :, 0:128], 1.0, None,
                                AL.mult)

        # A1_h = attn_h @ Wv'_h ; W_effT ; f2
        ps_a1 = pps.tile([128, 2, 512], F32, tag="ps")
        for hh in range(2):
            nc.tensor.matmul(out=ps_a1[:, hh, 0:256], lhsT=attT[:, hh, :],
                             rhs=wv_sb[:, hh, :], start=True, stop=True)
            nc.tensor.matmul(out=ps_a1[:, hh, 256:257], lhsT=attT[:, hh, :],
                             rhs=bv_col[:, hh:hh + 1], start=True, stop=True)
        a1 = post.tile([128, 2, 256], BF, tag="a1")
        nc.scalar.activation(out=a1[:], in_=ps_a1[:, :, 0:256],
                             func=AF.Identity, bias=zcol[:, :], scale=1.0)
        rc = small.tile([128, 2], BF, tag="rc")
        nc.vector.tensor_scalar(rc[:], ps_a1[:, :, 256], 1.0, None, AL.mult)

        ps_we = pps.tile([128, 2, 512], F32, tag="ps")
        for dch in range(2):
            for hh in range(2):
                nc.tensor.matmul(out=ps_we[:, dch, 0:256],
                                 lhsT=a1[:, hh, dch * 128:(dch + 1) * 128],
                                 rhs=woT[:, hh, :], start=(hh == 0),
                                 stop=(hh == 1))
        weT = post.tile([128, 2, 256], BF, tag="weT")
        nc.scalar.activation(out=weT[:], in_=ps_we[:, :, 0:256],
                             func=AF.Identity, bias=zcol[:, :], scale=1.0)

        # f2 = Wo rc + bo (column), -> row -> broadcast tile
        ps_f2 = pps.tile([128, 2, 512], F32, tag="ps")
        for och in range(2):
            for hh in range(2):
                nc.tensor.matmul(
                    out=ps_f2[:, 0, och:och + 1],
                    lhsT=woT[:, hh, och * 128:(och + 1) * 128],
                    rhs=rc[:, hh:hh + 1], start=(hh == 0), stop=(hh == 1))
        f2c = small.tile([128, 2], F32, tag="f2c")
        nc.vector.scalar_tensor_tensor(out=f2c[:], in0=ps_f2[:, 0, 0:2],
                                       scalar=1.0, op0=AL.bypass, op1=AL.add,
                                       in1=bo_col[:])
        ps_f2r = pps.tile([128, 2, 512], F32, tag="ps")
        f2row = post.tile([1, 2, 128], BF, tag="f2row")
        for och in range(2):
            nc.tensor.transpose(ps_f2r[0:1, 0, och * 128:(och + 1) * 128],
                                f2c[:, och:och + 1], eyef[:])
            nc.vector.tensor_scalar(
                f2row[0:1, och, :],
                ps_f2r[0:1, 0, och * 128:(och + 1) * 128], 1.0, None, AL.mult)
        ps_fb = pps.tile([128, 2, 512], F32, tag="ps")
        for och in range(2):
            nc.tensor.matmul(out=ps_fb[:, 0, och * 128:(och + 1) * 128],
                             lhsT=ones_row[:], rhs=f2row[0:1, och, :],
                             start=True, stop=True)
        f2b = post.tile([128, 256], F32, tag="f2b")
        nc.vector.tensor_scalar(f2b[:], ps_fb[:, 0, 0:256], 1.0, None, AL.mult)

    # ================= phase 4: output pass =================
    f2b2 = post.tile([128, 2, 256], F32, tag="f2b2")
    nc.vector.tensor_scalar(f2b2[:, 0, :], f2b[:], 1.0, None, AL.mult)
    nc.vector.tensor_scalar(f2b2[:, 1, :], f2b[:], 1.0, None, AL.mult)
    with tc.tile_pool(name="ops", bufs=2, space="PSUM") as ops:
        for g in range(NT // 4):
            osb = outp.tile([128, 4, 256], BF, tag="osb")
            for h2 in range(2):
                op_ps = ops.tile([128, 2, 256], F32, tag="op")
                for q2 in range(2):
                    j = g * 4 + h2 * 2 + q2
                    for dch in range(2):
                        nc.tensor.matmul(out=op_ps[:, q2, :],
                                         lhsT=rdm[:, j, dch, :],
                                         rhs=weT[:, dch, :], start=(dch == 0),
                                         stop=(dch == 1))
                nc.vector.scalar_tensor_tensor(
                    out=osb[:, h2 * 2:h2 * 2 + 2, :], in0=op_ps[:], scalar=1.0,
                    op0=AL.bypass, op1=AL.add, in1=f2b2[:])
            nc.sync.dma_start(out=outv[:, g * 4:(g + 1) * 4, :], in_=osb[:])


# ======================= host side =======================

def _prep_shared(inputs):
    f32 = np.float32
    Wq = np.asarray(inputs["Wq"], f32)
    bq = np.asarray(inputs["bq"], f32)
    Wkv = np.asarray(inputs["Wkv"], f32)
    bkv = np.asarray(inputs["bkv"], f32)
    Wo = np.asarray(inputs["Wo"], f32)
    bo = np.asarray(inputs["bo"], f32)
    lnS_w = np.asarray(inputs["lnS_w"], f32)
    lnS_b = np.asarray(inputs["lnS_b"], f32)
    lnR_w = np.asarray(inputs["lnR_w"], f32)
    lnR_b = np.asarray(inputs["lnR_b"], f32)
    temp = np.asarray(inputs["temperature"], f32).reshape(H)

    Wk, Wv = Wkv[:D], Wkv[D:]
    Wqp = Wq * lnS_w[None, :]
    Wkp = Wk * lnR_w[None, :]
    Wvp = Wv * lnR_w[None, :]
    bq2 = Wq @ lnS_b + bq
    bk2 = Wk @ lnR_b + bkv[:D]
    bv2 = Wv @ lnR_b + bkv[D:]

    def colh(v, dt=f32):
        return np.ascontiguousarray(v.reshape(H, 128).T).astype(dt)

    rows = np.concatenate([bq2, bk2, 2.0 * bq2, 2.0 * bk2,
                           float(T) * bq2, float(T) * bk2]).reshape(1, 6 * D)
    return {
        "wqT": np.ascontiguousarray(Wqp.T).astype(BF16),
        "wkT": np.ascontiguousarray(Wkp.T).astype(BF16),
        "wv": np.ascontiguousarray(Wvp).astype(BF16),
        "woT": np.ascontiguousarray(Wo.T).astype(BF16),
        "rows": rows.astype(BF16),
        "bv_col": colh(bv2, BF16),
        "bo_col": colh(bo),
        "temp_col": np.broadcast_to(temp[None, :], (128, H)).astype(f32).copy(),
        "eyef": np.eye(128, dtype=f32),
        "eyeb": np.eye(128, dtype=f32).astype(BF16),
    }


def _get_nc():
    if "nc" not in _nc_cache:
        _nc_cache["nc"] = _build_nc()
    return _nc_cache["nc"]


def run(inputs, trace=False):
    nc = _get_nc()
    shared = _prep_shared(inputs)
    iR = np.asarray(inputs["input_R"], np.float32)
    iS = np.asarray(inputs["input_S"], np.float32)
    in_maps = []
    for ci in range(N_CORES):
        b, half = ci // 2, ci % 2
        m = dict(shared)
        m["x_r"] = np.ascontiguousarray(iR[b, half * T:(half + 1) * T])
        m["x_s"] = np.ascontiguousarray(iS[b, half * T:(half + 1) * T])
        in_maps.append(m)
    res = run_bass_kernel_spmd(nc, in_maps, list(range(N_CORES)), trace=trace)
    out = np.zeros((B, N, D), np.float32)
    for ci in range(N_CORES):
        b, half = ci // 2, ci % 2
        out[b, half * T:(half + 1) * T] = np.asarray(
            res.results[ci]["out"]).astype(np.float32)
    return out, res


def kernel(**inputs):
    out, _ = run(inputs, trace=False)
    return out


# revision 14
# speedup vs baseline: 1.0493x; 1.0493x over previous
"""Channel-attention (XCA-style) Trainium2 kernel, 8-way SPMD — v2.

Shapes (hardcoded): B=4, N=16384, D=256, H=2 heads, c=128.
Sharding: core ci -> batch b=ci//2, token half ci%2 (T=8192 tokens/core).

Covariance formulation: per core, exactly LayerNorm the bf16 token tiles
(one fused scale+bias op per tile), accumulate three 256x256 token-
contracted Grams (M_SS, M_RS, M_RR) plus channel sums on the PE, then
  G    = Wq' M_SR Wk'^T + rank-1 bias outer-products   (head-diag blocks)
  dq/dk = diag(Wq' M_SS Wq'^T) + bias terms            (eye-dot on PE out)
One pair AllReduce of [128, 260] (G | dq | dk). Post-collective, softmax
gives attn; attn@v and the output projection collapse into a single
256x256 effective weight W_eff = Wo . blockdiag(attn_h) . Wv', applied to
the (transposed) normalized R in one matmul pass; per-token work in the
output phase is a single PSUM+bias-row evacuation. Output lands bf16 in
DRAM; the host upcasts to fp32.
"""
import sys, types

sys.path.insert(0, "/opt/trn_rl_repo")

try:
    import antenv
    if "antenv.axon_hooks" not in sys.modules:
        _hooks = types.ModuleType("antenv.axon_hooks")
        _hooks._hook = None
        _hooks.set_axon_ntff_profile_hook = lambda h: setattr(_hooks, "_hook", h)
        _hooks.get_axon_ntff_profile_hook = lambda: _hooks._hook
        sys.modules["antenv.axon_hooks"] = _hooks
        antenv.axon_hooks = _hooks
        from trn_agent_boot.trn_boot import _ntff_profile_via_ctypes
        _hooks.set_axon_ntff_profile_hook(
            _ntff_profile_via_ctypes("/opt/axon/libaxon_pjrt.so"))
except Exception:
    pass

import numpy as np
import ml_dtypes

import concourse.bass as bass
import concourse.bacc as bacc
import concourse.mybir as mybir
import concourse.tile as tile
from concourse.bass_utils import run_bass_kernel_spmd

BF16 = ml_dtypes.bfloat16
F32 = mybir.dt.float32
BF = mybir.dt.bfloat16
AL = mybir.AluOpType
AF = mybir.ActivationFunctionType
AX = mybir.AxisListType

B, N, D, H = 4, 16384, 256, 2
C = D // H
T = N // 2                 # tokens per core
NT = T // 128              # 64 token tiles / core
EPS_LN = 1e-5
EPS_NORM = 1e-12
N_CORES = 8
CHT = 4                    # token tiles per chunk
NCH = NT // CHT            # 8 chunks
PAYW = 260                 # collective payload width (G 256 | dq 2 | dk 2)

_nc_cache = {}


def _build_nc():
    nc = bacc.Bacc("TRN2", target_bir_lowering=False, debug=False,
                   num_devices=N_CORES)

    def ein(name, shape, dt=F32):
        return nc.dram_tensor(name, list(shape), dt, kind="ExternalInput")

    d_s = ein("x_s", [T, D])            # q source shard (input_S)
    d_r = ein("x_r", [T, D])            # kv source shard (input_R)
    d_wqT = ein("wqT", [D, D], BF)      # Wq'(=Wq.diag(lnS_w)) transposed [e,c]
    d_wkT = ein("wkT", [D, D], BF)
    d_wv = ein("wv", [D, D], BF)        # Wv' natural [c, e]
    d_woT = ein("woT", [D, D], BF)      # Wo transposed [c, o]
    d_rows = ein("rows", [1, 6 * D], BF)  # bq|bk|2bq|2bk|T*bq|T*bk rows
    d_bv = ein("bv_col", [128, H], BF)
    d_bo = ein("bo_col", [128, H])
    d_temp = ein("temp_col", [128, H])
    d_eyef = ein("eyef", [128, 128])
    d_eyeb = ein("eyeb", [128, 128], BF)
    d_out = nc.dram_tensor("out", [T, D], BF, kind="ExternalOutput")

    sv = d_s.rearrange("(j p) d -> p j d", p=128)
    rv = d_r.rearrange("(j p) d -> p j d", p=128)
    outv = d_out.rearrange("(j p) d -> p j d", p=128)

    with tile.TileContext(nc) as tc:
        import contextlib
        with contextlib.ExitStack() as ctx:
            _body(ctx, tc, nc, sv, rv, outv, d_wqT, d_wkT, d_wv, d_woT,
                  d_rows, d_bv, d_bo, d_temp, d_eyef, d_eyeb)
    nc.finalize()
    return nc


def _body(ctx, tc, nc, sv, rv, outv, d_wqT, d_wkT, d_wv, d_woT, d_rows,
          d_bv, d_bo, d_temp, d_eyef, d_eyeb):
    E = ctx.enter_context
    consts = E(tc.tile_pool(name="consts", bufs=1))
    stage = E(tc.tile_pool(name="stage", bufs=5))
    scrp = E(tc.tile_pool(name="scrp", bufs=2))
    nrm = E(tc.tile_pool(name="nrm", bufs=3))
    stp = E(tc.tile_pool(name="stp", bufs=3))
    pers = E(tc.tile_pool(name="pers", bufs=1))
    post = E(tc.tile_pool(name="post", bufs=1))
    small = E(tc.tile_pool(name="small", bufs=4))
    outp = E(tc.tile_pool(name="outp", bufs=2))
    dram = E(tc.tile_pool(name="dram", bufs=1, space="DRAM"))
    gacc = E(tc.tile_pool(name="gacc", bufs=1, space="PSUM"))

    # ---------------- constants ----------------
    wqT = consts.tile([128, 2, D], BF, tag="wqT")
    wkT = consts.tile([128, 2, D], BF, tag="wkT")
    wv_sb = consts.tile([128, 2, D], BF, tag="wv")
    woT = consts.tile([128, 2, D], BF, tag="woT")
    for dst, src in ((wqT, d_wqT), (wkT, d_wkT), (wv_sb, d_wv), (woT, d_woT)):
        nc.sync.dma_start(out=dst[:], in_=src.rearrange("(h p) o -> p h o", p=128))
    rows_sb = consts.tile([1, 6 * D], BF, tag="rows")
    nc.sync.dma_start(out=rows_sb[:], in_=d_rows[:, :])
    bq_row = rows_sb[0:1, 0 * D:1 * D]
    bk_row = rows_sb[0:1, 1 * D:2 * D]
    bq2_row = rows_sb[0:1, 2 * D:3 * D]
    bk2_row = rows_sb[0:1, 3 * D:4 * D]
    bqT_row = rows_sb[0:1, 4 * D:5 * D]
    bkT_row = rows_sb[0:1, 5 * D:6 * D]
    bv_col = consts.tile([128, H], BF, tag="bv")
    bo_col = consts.tile([128, H], F32, tag="bo")
    temp_col = consts.tile([128, H], F32, tag="temp")
    for dst, src in ((bv_col, d_bv), (bo_col, d_bo), (temp_col, d_temp)):
        nc.sync.dma_start(out=dst[:], in_=src[:, :])
    eyef = consts.tile([128, 128], F32, tag="eyef")
    eyeb = consts.tile([128, 128], BF, tag="eyeb")
    nc.sync.dma_start(out=eyef[:], in_=d_eyef[:, :])
    ones_col = consts.tile([128, 1], BF, tag="ones_c")
    nc.vector.memset(ones_col[:], 1.0)
    ones_row = consts.tile([1, 128], BF, tag="ones_r")
    nc.gpsimd.memset(ones_row[:], 1.0)
    epsln = consts.tile([128, 1], F32, tag="epsln")
    nc.vector.memset(epsln[:], EPS_LN)
    zcol = consts.tile([128, 1], F32, tag="zcol")
    nc.vector.memset(zcol[:], 0.0)

    rdm = pers.tile([128, NT, 2, 128], BF, tag="rdm")   # normalized R, d-major

    psSS = gacc.tile([128, 2, 256], F32, tag="psSS")
    psRX = gacc.tile([128, 2, 512], F32, tag="psRX")    # [M_RS | M_RR] blocks
    psSum = gacc.tile([128, 512], F32, tag="psSum")     # row 0: [s_S | s_R]

    # ================= phase 1: software-pipelined chunk stream ==========
    rawt = [None] * NCH
    sqt = [None] * NCH
    s1t = [None] * NCH
    s2t = [None] * NCH
    at_ = [None] * NCH
    bt_ = [None] * NCH
    ntt = [None] * NCH

    def p1_load(c):
        j0 = c * CHT
        raw = stage.tile([128, 2, CHT, 256], BF, tag="raw")
        nc.gpsimd.dma_start(out=raw[:, 0], in_=sv[:, j0:j0 + CHT, :])
        nc.gpsimd.dma_start(out=raw[:, 1], in_=rv[:, j0:j0 + CHT, :])
        rawt[c] = raw

    def p1_stats_a(c):
        raw = rawt[c]
        sq = scrp.tile([128, 2, CHT, 256], BF, tag="sq")
        nc.scalar.activation(out=sq[:], in_=raw[:], func=AF.Square,
                             bias=zcol[:, :], scale=1.0)
        s1 = stp.tile([128, 2, CHT], F32, tag="s1")
        nc.vector.tensor_reduce(out=s1[:], in_=raw[:], axis=AX.X, op=AL.add)
        sqt[c], s1t[c] = sq, s1

    def p1_stats_b(c):
        sq, s1 = sqt[c], s1t[c]
        s2 = stp.tile([128, 2, CHT], F32, tag="s2")
        nc.vector.tensor_reduce(out=s2[:], in_=sq[:], axis=AX.X, op=AL.add)
        p = stp.tile([128, 2, CHT], F32, tag="p")
        var = stp.tile([128, 2, CHT], F32, tag="var")
        sig = stp.tile([128, 2, CHT], F32, tag="sig")
        # var = s2 - s1^2/D ; sigma = sqrt(var/D + eps) via ACT scale
        nc.vector.scalar_tensor_tensor(out=p[:], in0=s1[:], scalar=-1.0 / D,
                                       op0=AL.mult, op1=AL.mult, in1=s1[:])
        nc.vector.scalar_tensor_tensor(out=var[:], in0=s2[:], scalar=1.0,
                                       op0=AL.bypass, op1=AL.add, in1=p[:])
        nc.scalar.activation(out=sig[:], in_=var[:], func=AF.Sqrt,
                             bias=epsln[:, :], scale=1.0 / D)
        a_sc = stp.tile([128, 2, CHT], F32, tag="a_sc")
        b_sc = stp.tile([128, 2, CHT], F32, tag="b_sc")
        nc.vector.reciprocal(out=a_sc[:], in_=sig[:])
        nc.vector.scalar_tensor_tensor(out=b_sc[:], in0=s1[:], scalar=-1.0 / D,
                                       op0=AL.mult, op1=AL.mult, in1=a_sc[:])
        at_[c], bt_[c] = a_sc, b_sc

    def p1_norms(c):
        raw, a_sc, b_sc = rawt[c], at_[c], bt_[c]
        nt = nrm.tile([128, 2, CHT, 256], BF, tag="nt")
        # balance: DVE gets 2 tiles, ACT gets 6
        for jj in range(CHT):
            if jj < 2:
                nc.vector.tensor_scalar(nt[:, 0, jj], raw[:, 0, jj],
                                        a_sc[:, 0, jj:jj + 1],
                                        b_sc[:, 0, jj:jj + 1], AL.mult, AL.add)
            else:
                nc.scalar.activation(out=nt[:, 0, jj], in_=raw[:, 0, jj],
                                     func=AF.Identity,
                                     bias=b_sc[:, 0, jj:jj + 1],
                                     scale=a_sc[:, 0, jj:jj + 1])
            nc.scalar.activation(out=nt[:, 1, jj], in_=raw[:, 1, jj],
                                 func=AF.Identity, bias=b_sc[:, 1, jj:jj + 1],
                                 scale=a_sc[:, 1, jj:jj + 1])
        ntt[c] = nt

    def p1_mms(c):
        j0 = c * CHT
        nt = ntt[c]
        for jj in range(CHT):
            j = j0 + jj
            st = (j == 0)
            sp = (j == NT - 1)
            for ech in range(2):
                nc.tensor.matmul(out=psSS[:, ech, :],
                                 lhsT=nt[:, 0, jj, ech * 128:(ech + 1) * 128],
                                 rhs=nt[:, 0, jj, :], start=st, stop=sp)
            for fch in range(2):
                nc.tensor.matmul(out=psRX[:, fch, :],
                                 lhsT=nt[:, 1, jj, fch * 128:(fch + 1) * 128],
                                 rhs=nt[:, :, jj, :], start=st, stop=sp)
            nc.tensor.matmul(out=psSum[0:1, :], lhsT=ones_col[:],
                             rhs=nt[:, :, jj, :], start=st, stop=sp)
        nc.sync.dma_start_transpose(rdm[:, j0:j0 + CHT, :, :], nt[:, 1])

    for i in range(NCH + 3):
        if i < NCH:
            p1_load(i)
        if 2 <= i < NCH + 2:
            p1_stats_b(i - 2)
        if 1 <= i <= NCH:
            p1_stats_a(i - 1)
        if i >= 3:
            p1_norms(i - 3)
            p1_mms(i - 3)

    # ================= phase 2: local reductions -> payload ================
    mSS = post.tile([128, 2, 256], BF, tag="mSS")
    mRS = post.tile([128, 2, 256], BF, tag="mRS")
    mRR = post.tile([128, 2, 256], BF, tag="mRR")
    nc.vector.tensor_scalar(mSS[:], psSS[:], 1.0, None, AL.mult)
    nc.scalar.activation(out=mRS[:], in_=psRX[:, :, 0:256], func=AF.Identity,
                         bias=zcol[:, :], scale=1.0)
    nc.scalar.activation(out=mRR[:], in_=psRX[:, :, 256:512], func=AF.Identity,
                         bias=zcol[:, :], scale=1.0)
    sums_sb = post.tile([1, 512], F32, tag="sums")
    nc.vector.tensor_scalar(sums_sb[:], psSum[0:1, :], 1.0, None, AL.mult)

    with tc.tile_pool(name="pps", bufs=2, space="PSUM") as pps:
        # s_S / s_R rows -> bf16 columns via PE transpose
        ps_sc = pps.tile([128, 2, 512], F32, tag="ps")
        for k in range(4):
            nc.tensor.transpose(ps_sc[:, 0, k:k + 1],
                                sums_sb[0:1, k * 128:(k + 1) * 128],
                                eyef[0:1, 0:1])
        scol = post.tile([128, 4], BF, tag="scol")   # sS e0,e1 | sR e0,e1
        nc.vector.tensor_scalar(scol[:], ps_sc[:, 0, 0:4], 1.0, None, AL.mult)

        # (Wq' s_S) and (Wk' s_R) as rows [1, 256]
        ps_r = pps.tile([128, 2, 512], F32, tag="ps")
        for ech in range(2):
            nc.tensor.matmul(out=ps_r[0:1, 0, 0:256], lhsT=scol[:, ech:ech + 1],
                             rhs=wqT[:, ech, :], start=(ech == 0),
                             stop=(ech == 1))
            nc.tensor.matmul(out=ps_r[0:1, 0, 256:512],
                             lhsT=scol[:, 2 + ech:3 + ech],
                             rhs=wkT[:, ech, :], start=(ech == 0),
                             stop=(ech == 1))
        prows = post.tile([1, 512], BF, tag="prows")  # wqss row | wksr row
        nc.vector.tensor_scalar(prows[:], ps_r[0:1, 0, :], 1.0, None, AL.mult)
        wqss_row = prows[0:1, 0:256]
        wksr_row = prows[0:1, 256:512]

        # V = M_SR Wk'^T  (lhsT = M_RS blocks)
        ps_v = pps.tile([128, 2, 512], F32, tag="ps")
        for ech in range(2):
            for fch in range(2):
                nc.tensor.matmul(out=ps_v[:, ech, 0:256],
                                 lhsT=mRS[:, fch, ech * 128:(ech + 1) * 128],
                                 rhs=wkT[:, fch, :], start=(fch == 0),
                                 stop=(fch == 1))
        v_sb = post.tile([128, 2, 256], BF, tag="v_sb")
        nc.scalar.activation(out=v_sb[:], in_=ps_v[:, :, 0:256],
                             func=AF.Identity, bias=zcol[:, :], scale=1.0)

        # Z_q = M_SS Wq'^T-ish, Z_k = M_RR Wk'^T (for norm diagonals)
        ps_z = pps.tile([128, 2, 512], F32, tag="ps")
        for ech in range(2):
            for fch in range(2):
                nc.tensor.matmul(out=ps_z[:, ech, 0:256],
                                 lhsT=mSS[:, fch, ech * 128:(ech + 1) * 128],
                                 rhs=wqT[:, fch, :], start=(fch == 0),
                                 stop=(fch == 1))
                nc.tensor.matmul(out=ps_z[:, ech, 256:512],
                                 lhsT=mRR[:, fch, ech * 128:(ech + 1) * 128],
                                 rhs=wkT[:, fch, :], start=(fch == 0),
                                 stop=(fch == 1))
        z_sb = post.tile([128, 2, 512], BF, tag="z_sb")
        nc.vector.tensor_scalar(z_sb[:], ps_z[:], 1.0, None, AL.mult)

        payload = post.tile([128, PAYW], F32, tag="payload")

        # G head blocks + bias outers
        ps_g = pps.tile([128, 2, 512], F32, tag="ps")
        for hh in range(2):
            g = ps_g[:, 0, hh * 128:(hh + 1) * 128]
            hs = slice(hh * 128, (hh + 1) * 128)
            for ech in range(2):
                nc.tensor.matmul(out=g, lhsT=wqT[:, ech, hs],
                                 rhs=v_sb[:, ech, hs], start=(ech == 0),
                                 stop=False)
            nc.tensor.matmul(out=g, lhsT=bq_row[:, hs], rhs=wksr_row[:, hs],
                             start=False, stop=False)
            nc.tensor.matmul(out=g, lhsT=wqss_row[:, hs], rhs=bk_row[:, hs],
                             start=False, stop=False)
            nc.tensor.matmul(out=g, lhsT=bq_row[:, hs], rhs=bkT_row[:, hs],
                             start=False, stop=True)
        nc.vector.tensor_scalar(payload[:, 0:256], ps_g[:, 0, 0:256], 1.0,
                                None, AL.mult)

        # Y_q / Y_k head blocks + bias outers; eye-dot -> dq, dk
        ps_y = pps.tile([128, 2, 512], F32, tag="ps")
        for hh in range(2):
            hs = slice(hh * 128, (hh + 1) * 128)
            yq = ps_y[:, hh, 0:128]
            yk = ps_y[:, hh, 128:256]
            for ech in range(2):
                nc.tensor.matmul(out=yq, lhsT=wqT[:, ech, hs],
                                 rhs=z_sb[:, ech, hs], start=(ech == 0),
                                 stop=False)
            nc.tensor.matmul(out=yq, lhsT=bq2_row[:, hs], rhs=wqss_row[:, hs],
                             start=False, stop=False)
            nc.tensor.matmul(out=yq, lhsT=bq_row[:, hs], rhs=bqT_row[:, hs],
                             start=False, stop=True)
            for ech in range(2):
                nc.tensor.matmul(
                    out=yk, lhsT=wkT[:, ech, hs],
                    rhs=z_sb[:, ech, 256 + hh * 128:256 + (hh + 1) * 128],
                    start=(ech == 0), stop=False)
            nc.tensor.matmul(out=yk, lhsT=bk2_row[:, hs], rhs=wksr_row[:, hs],
                             start=False, stop=False)
            nc.tensor.matmul(out=yk, lhsT=bk_row[:, hs], rhs=bkT_row[:, hs],
                             start=False, stop=True)
            dscr = small.tile([128, 128], F32, tag="dscr")
            nc.vector.scalar_tensor_tensor(
                out=dscr[:], in0=yq, scalar=1.0, op0=AL.mult, op1=AL.mult,
                in1=eyef[:], accum_out=payload[:, 256 + hh:257 + hh])
            nc.vector.scalar_tensor_tensor(
                out=dscr[:], in0=yk, scalar=1.0, op0=AL.mult, op1=AL.mult,
                in1=eyef[:], accum_out=payload[:, 258 + hh:259 + hh])

        # ---------------- collective ----------------
        cc_in = dram.tile([128, PAYW], F32)
        cc_out = dram.tile([128, PAYW], F32)
        nc.gpsimd.dma_start(out=cc_in[:, :], in_=payload[:])
        nc.gpsimd.collective_compute(
            "AllReduce", AL.add,
            replica_groups=[[0, 1], [2, 3], [4, 5], [6, 7]],
            ins=[cc_in.opt()], outs=[cc_out.opt()])
        # keep the PE HAM-warm through the collective window: harmless
        # 512-wide matmuls into the retired psSum bank (~14 us of busy)
        for _ in range(48):
            nc.tensor.matmul(out=psSum[:, :], lhsT=wqT[:, 0, 0:128],
                             rhs=wv_sb[:, 0:2, 0:256], start=True, stop=True)
        red = post.tile([128, PAYW], F32, tag="red")
        nc.sync.dma_start(out=red[:], in_=cc_out[:, :])

        # ---------------- phase 3: softmax + W_eff ----------------
        nrmc = small.tile([128, 4], F32, tag="nrmc")
        nc.scalar.activation(out=nrmc[:], in_=red[:, 256:260], func=AF.Sqrt,
                             bias=zcol[:, :], scale=1.0)
        nc.vector.tensor_scalar_max(nrmc[:], nrmc[:], EPS_NORM)
        nc.vector.reciprocal(out=nrmc[:], in_=nrmc[:])
        iq = small.tile([128, 2], F32, tag="iq")
        nc.vector.tensor_tensor(out=iq[:], in0=nrmc[:, 0:2], in1=temp_col[:],
                                op=AL.mult)

        # invk column -> broadcast tile via PE transpose + outer product
        ps_t = pps.tile([128, 2, 512], F32, tag="ps")
        ikrow = post.tile([1, 2, 128], BF, tag="ikrow")
        for hh in range(2):
            nc.tensor.transpose(ps_t[0:1, 0, hh * 128:(hh + 1) * 128],
                                nrmc[:, 2 + hh:3 + hh], eyef[:])
            nc.vector.tensor_scalar(
                ikrow[0:1, hh, :], ps_t[0:1, 0, hh * 128:(hh + 1) * 128],
                1.0, None, AL.mult)
        ps_ik = pps.tile([128, 2, 512], F32, tag="ps")
        for hh in range(2):
            nc.tensor.matmul(out=ps_ik[:, 0, hh * 128:(hh + 1) * 128],
                             lhsT=ones_row[:], rhs=ikrow[0:1, hh, :],
                             start=True, stop=True)
        ikb = post.tile([128, 2, 128], F32, tag="ikb")
        nc.vector.tensor_scalar(ikb[:], ps_ik[:, 0, 0:256], 1.0, None, AL.mult)

        # logits, softmax
        lg = post.tile([128, 2, 128], F32, tag="lg")
        for hh in range(2):
            nc.vector.tensor_scalar(lg[:, hh, :], red[:, hh * 128:(hh + 1) * 128],
                                    iq[:, hh:hh + 1], None, AL.mult)
        nc.vector.tensor_tensor(out=lg[:], in0=lg[:], in1=ikb[:], op=AL.mult)
        rmax = small.tile([128, 2], F32, tag="rmax")
        nc.vector.tensor_reduce(out=rmax[:], in_=lg[:], axis=AX.X, op=AL.max)
        nc.vector.tensor_scalar(rmax[:], rmax[:], -1.0, None, AL.mult)
        att = post.tile([128, 2, 128], F32, tag="att")
        for hh in range(2):
            nc.scalar.activation(out=att[:, hh, :], in_=lg[:, hh, :],
                                 func=AF.Exp, bias=rmax[:, hh:hh + 1],
                                 scale=1.0)
        rs = small.tile([128, 2], F32, tag="rs")
        nc.vector.tensor_reduce(out=rs[:], in_=att[:], axis=AX.X, op=AL.add)
        nc.vector.reciprocal(out=rs[:], in_=rs[:])
        attf = post.tile([128, 2, 128], F32, tag="attf")
        for hh in range(2):
            nc.vector.tensor_scalar(attf[:, hh, :], att[:, hh, :],
                                    rs[:, hh:hh + 1], None, AL.mult)

        # attn^T
        ps_at = pps.tile([128, 2, 512], F32, tag="ps")
        for hh in range(2):
            nc.tensor.transpose(ps_at[:, hh, 0:128], attf[:, hh, :], eyef[:])
        attT = post.tile([128, 2, 128], BF, tag="attT")
        nc.vector.tensor_scalar(attT[:], ps_at[:, :, 0:128], 1.0, None,
                                AL.mult)

        # A1_h = attn_h @ Wv'_h ; W_effT ; f2
        ps_a1 = pps.tile([128, 2, 512], F32, tag="ps")
        for hh in range(2):
            nc.tensor.matmul(out=ps_a1[:, hh, 0:256], lhsT=attT[:, hh, :],
                             rhs=wv_sb[:, hh, :], start=True, stop=True)
            nc.tensor.matmul(out=ps_a1[:, hh, 256:257], lhsT=attT[:, hh, :],
                             rhs=bv_col[:, hh:hh + 1], start=True, stop=True)
        a1 = post.tile([128, 2, 256], BF, tag="a1")
        nc.scalar.activation(out=a1[:], in_=ps_a1[:, :, 0:256],
                             func=AF.Identity, bias=zcol[:, :], scale=1.0)
        rc = small.tile([128, 2], BF, tag="rc")
        nc.vector.tensor_scalar(rc[:], ps_a1[:, :, 256], 1.0, None, AL.mult)

        ps_we = pps.tile([128, 2, 512], F32, tag="ps")
        for dch in range(2):
            for hh in range(2):
                nc.tensor.matmul(out=ps_we[:, dch, 0:256],
                                 lhsT=a1[:, hh, dch * 128:(dch + 1) * 128],
                                 rhs=woT[:, hh, :], start=(hh == 0),
                                 stop=(hh == 1))
        weT = post.tile([128, 2, 256], BF, tag="weT")
        nc.scalar.activation(out=weT[:], in_=ps_we[:, :, 0:256],
                             func=AF.Identity, bias=zcol[:, :], scale=1.0)

        # f2 = Wo rc + bo (column), -> row -> broadcast tile
        ps_f2 = pps.tile([128, 2, 512], F32, tag="ps")
        for och in range(2):
            for hh in range(2):
                nc.tensor.matmul(
                    out=ps_f2[:, 0, och:och + 1],
                    lhsT=woT[:, hh, och * 128:(och + 1) * 128],
                    rhs=rc[:, hh:hh + 1], start=(hh == 0), stop=(hh == 1))
        f2c = small.tile([128, 2], F32, tag="f2c")
        nc.vector.scalar_tensor_tensor(out=f2c[:], in0=ps_f2[:, 0, 0:2],
                                       scalar=1.0, op0=AL.bypass, op1=AL.add,
                                       in1=bo_col[:])
        ps_f2r = pps.tile([128, 2, 512], F32, tag="ps")
        f2row = post.tile([1, 2, 128], BF, tag="f2row")
        for och in range(2):
            nc.tensor.transpose(ps_f2r[0:1, 0, och * 128:(och + 1) * 128],
                                f2c[:, och:och + 1], eyef[:])
            nc.vector.tensor_scalar(
                f2row[0:1, och, :],
                ps_f2r[0:1, 0, och * 128:(och + 1) * 128], 1.0, None, AL.mult)
        ps_fb = pps.tile([128, 2, 512], F32, tag="ps")
        for och in range(2):
            nc.tensor.matmul(out=ps_fb[:, 0, och * 128:(och + 1) * 128],
                             lhsT=ones_row[:], rhs=f2row[0:1, och, :],
                             start=True, stop=True)
        f2b = post.tile([128, 256], F32, tag="f2b")
        nc.vector.tensor_scalar(f2b[:], ps_fb[:, 0, 0:256], 1.0, None, AL.mult)

    # ================= phase 4: output pass =================
    f2b2 = post.tile([128, 2, 256], F32, tag="f2b2")
    nc.vector.tensor_scalar(f2b2[:, 0, :], f2b[:], 1.0, None, AL.mult)
    nc.vector.tensor_scalar(f2b2[:, 1, :], f2b[:], 1.0, None, AL.mult)
    with tc.tile_pool(name="ops", bufs=2, space="PSUM") as ops:
        for g in range(NT // 4):
            osb = outp.tile([128, 4, 256], BF, tag="osb")
            for h2 in range(2):
                op_ps = ops.tile([128, 2, 256], F32, tag="op")
                for q2 in range(2):
                    j = g * 4 + h2 * 2 + q2
                    for dch in range(2):
                        nc.tensor.matmul(out=op_ps[:, q2, :],
                                         lhsT=rdm[:, j, dch, :],
                                         rhs=weT[:, dch, :], start=(dch == 0),
                                         stop=(dch == 1))
                nc.vector.scalar_tensor_tensor(
                    out=osb[:, h2 * 2:h2 * 2 + 2, :], in0=op_ps[:], scalar=1.0,
                    op0=AL.bypass, op1=AL.add, in1=f2b2[:])
            nc.sync.dma_start(out=outv[:, g * 4:(g + 1) * 4, :], in_=osb[:])


# ======================= host side =======================

def _prep_shared(inputs):
    f32 = np.float32
    Wq = np.asarray(inputs["Wq"], f32)
    bq = np.asarray(inputs["bq"], f32)
    Wkv = np.asarray(inputs["Wkv"], f32)
    bkv = np.asarray(inputs["bkv"], f32)
    Wo = np.asarray(inputs["Wo"], f32)
    bo = np.asarray(inputs["bo"], f32)
    lnS_w = np.asarray(inputs["lnS_w"], f32)
    lnS_b = np.asarray(inputs["lnS_b"], f32)
    lnR_w = np.asarray(inputs["lnR_w"], f32)
    lnR_b = np.asarray(inputs["lnR_b"], f32)
    temp = np.asarray(inputs["temperature"], f32).reshape(H)

    Wk, Wv = Wkv[:D], Wkv[D:]
    Wqp = Wq * lnS_w[None, :]
    Wkp = Wk * lnR_w[None, :]
    Wvp = Wv * lnR_w[None, :]
    bq2 = Wq @ lnS_b + bq
    bk2 = Wk @ lnR_b + bkv[:D]
    bv2 = Wv @ lnR_b + bkv[D:]

    def colh(v, dt=f32):
        return np.ascontiguousarray(v.reshape(H, 128).T).astype(dt)

    rows = np.concatenate([bq2, bk2, 2.0 * bq2, 2.0 * bk2,
                           float(T) * bq2, float(T) * bk2]).reshape(1, 6 * D)
    return {
        "wqT": np.ascontiguousarray(Wqp.T).astype(BF16),
        "wkT": np.ascontiguousarray(Wkp.T).astype(BF16),
        "wv": np.ascontiguousarray(Wvp).astype(BF16),
        "woT": np.ascontiguousarray(Wo.T).astype(BF16),
        "rows": rows.astype(BF16),
        "bv_col": colh(bv2, BF16),
        "bo_col": colh(bo),
        "temp_col": np.broadcast_to(temp[None, :], (128, H)).astype(f32).copy(),
        "eyef": np.eye(128, dtype=f32),
        "eyeb": np.eye(128, dtype=f32).astype(BF16),
    }


def _get_nc():
    if "nc" not in _nc_cache:
        _nc_cache["nc"] = _build_nc()
    return _nc_cache["nc"]


def run(inputs, trace=False):
    nc = _get_nc()
    shared = _prep_shared(inputs)
    iR = np.asarray(inputs["input_R"], np.float32)
    iS = np.asarray(inputs["input_S"], np.float32)
    in_maps = []
    for ci in range(N_CORES):
        b, half = ci // 2, ci % 2
        m = dict(shared)
        m["x_r"] = np.ascontiguousarray(iR[b, half * T:(half + 1) * T])
        m["x_s"] = np.ascontiguousarray(iS[b, half * T:(half + 1) * T])
        in_maps.append(m)
    res = run_bass_kernel_spmd(nc, in_maps, list(range(N_CORES)), trace=trace)
    out = np.zeros((B, N, D), np.float32)
    for ci in range(N_CORES):
        b, half = ci // 2, ci % 2
        out[b, half * T:(half + 1) * T] = np.asarray(
            res.results[ci]["out"]).astype(np.float32)
    return out, res


def kernel(**inputs):
    out, _ = run(inputs, trace=False)
    return out


# revision 15
# speedup vs baseline: 1.0702x; 1.0199x over previous
"""Channel-attention (XCA-style) Trainium2 kernel, 8-way SPMD — v2.

Shapes (hardcoded): B=4, N=16384, D=256, H=2 heads, c=128.
Sharding: core ci -> batch b=ci//2, token half ci%2 (T=8192 tokens/core).

Covariance formulation: per core, exactly LayerNorm the bf16 token tiles
(one fused scale+bias op per tile), accumulate three 256x256 token-
contracted Grams (M_SS, M_RS, M_RR) plus channel sums on the PE, then
  G    = Wq' M_SR Wk'^T + rank-1 bias outer-products   (head-diag blocks)
  dq/dk = diag(Wq' M_SS Wq'^T) + bias terms            (eye-dot on PE out)
One pair AllReduce of [128, 260] (G | dq | dk). Post-collective, softmax
gives attn; attn@v and the output projection collapse into a single
256x256 effective weight W_eff = Wo . blockdiag(attn_h) . Wv', applied to
the (transposed) normalized R in one matmul pass; per-token work in the
output phase is a single PSUM+bias-row evacuation. Output lands bf16 in
DRAM; the host upcasts to fp32.
"""
import sys, types

sys.path.insert(0, "/opt/trn_rl_repo")

try:
    import antenv
    if "antenv.axon_hooks" not in sys.modules:
        _hooks = types.ModuleType("antenv.axon_hooks")
        _hooks._hook = None
        _hooks.set_axon_ntff_profile_hook = lambda h: setattr(_hooks, "_hook", h)
        _hooks.get_axon_ntff_profile_hook = lambda: _hooks._hook
        sys.modules["antenv.axon_hooks"] = _hooks
        antenv.axon_hooks = _hooks
        from trn_agent_boot.trn_boot import _ntff_profile_via_ctypes
        _hooks.set_axon_ntff_profile_hook(
            _ntff_profile_via_ctypes("/opt/axon/libaxon_pjrt.so"))
except Exception:
    pass

import numpy as np
import ml_dtypes

import concourse.bass as bass
import concourse.bacc as bacc
import concourse.mybir as mybir
import concourse.tile as tile
from concourse.bass_utils import run_bass_kernel_spmd

BF16 = ml_dtypes.bfloat16
F32 = mybir.dt.float32
BF = mybir.dt.bfloat16
AL = mybir.AluOpType
AF = mybir.ActivationFunctionType
AX = mybir.AxisListType

B, N, D, H = 4, 16384, 256, 2
C = D // H
T = N // 2                 # tokens per core
NT = T // 128              # 64 token tiles / core
EPS_LN = 1e-5
EPS_NORM = 1e-12
N_CORES = 8
CHT = 4                    # token tiles per chunk
NCH = NT // CHT            # 8 chunks
PAYW = 260                 # collective payload width (G 256 | dq 2 | dk 2)

_nc_cache = {}


def _build_nc():
    nc = bacc.Bacc("TRN2", target_bir_lowering=False, debug=False,
                   num_devices=N_CORES)

    def ein(name, shape, dt=F32):
        return nc.dram_tensor(name, list(shape), dt, kind="ExternalInput")

    d_s = ein("x_s", [T, D])            # q source shard (input_S)
    d_r = ein("x_r", [T, D])            # kv source shard (input_R)
    d_wqT = ein("wqT", [D, D], BF)      # Wq'(=Wq.diag(lnS_w)) transposed [e,c]
    d_wkT = ein("wkT", [D, D], BF)
    d_wv = ein("wv", [D, D], BF)        # Wv' natural [c, e]
    d_woT = ein("woT", [D, D], BF)      # Wo transposed [c, o]
    d_rows = ein("rows", [1, 6 * D], BF)  # bq|bk|2bq|2bk|T*bq|T*bk rows
    d_bv = ein("bv_col", [128, H], BF)
    d_bo = ein("bo_col", [128, H])
    d_temp = ein("temp_col", [128, H])
    d_eyef = ein("eyef", [128, 128])
    d_eyeb = ein("eyeb", [128, 128], BF)
    d_out = nc.dram_tensor("out", [T, D], BF, kind="ExternalOutput")

    sv = d_s.rearrange("(j p) d -> p j d", p=128)
    rv = d_r.rearrange("(j p) d -> p j d", p=128)
    outv = d_out.rearrange("(j p) d -> p j d", p=128)

    with tile.TileContext(nc) as tc:
        import contextlib
        with contextlib.ExitStack() as ctx:
            _body(ctx, tc, nc, sv, rv, outv, d_wqT, d_wkT, d_wv, d_woT,
                  d_rows, d_bv, d_bo, d_temp, d_eyef, d_eyeb)
    nc.finalize()
    return nc


def _body(ctx, tc, nc, sv, rv, outv, d_wqT, d_wkT, d_wv, d_woT, d_rows,
          d_bv, d_bo, d_temp, d_eyef, d_eyeb):
    E = ctx.enter_context
    consts = E(tc.tile_pool(name="consts", bufs=1))
    stage = E(tc.tile_pool(name="stage", bufs=5))
    scrp = E(tc.tile_pool(name="scrp", bufs=2))
    nrm = E(tc.tile_pool(name="nrm", bufs=3))
    stp = E(tc.tile_pool(name="stp", bufs=3))
    pers = E(tc.tile_pool(name="pers", bufs=1))
    post = E(tc.tile_pool(name="post", bufs=1))
    small = E(tc.tile_pool(name="small", bufs=4))
    outp = E(tc.tile_pool(name="outp", bufs=2))
    dram = E(tc.tile_pool(name="dram", bufs=1, space="DRAM"))
    gacc = E(tc.tile_pool(name="gacc", bufs=1, space="PSUM"))

    # ---------------- constants ----------------
    wqT = consts.tile([128, 2, D], BF, tag="wqT")
    wkT = consts.tile([128, 2, D], BF, tag="wkT")
    wv_sb = consts.tile([128, 2, D], BF, tag="wv")
    woT = consts.tile([128, 2, D], BF, tag="woT")
    for dst, src in ((wqT, d_wqT), (wkT, d_wkT), (wv_sb, d_wv), (woT, d_woT)):
        nc.sync.dma_start(out=dst[:], in_=src.rearrange("(h p) o -> p h o", p=128))
    rows_sb = consts.tile([1, 6 * D], BF, tag="rows")
    nc.sync.dma_start(out=rows_sb[:], in_=d_rows[:, :])
    bq_row = rows_sb[0:1, 0 * D:1 * D]
    bk_row = rows_sb[0:1, 1 * D:2 * D]
    bq2_row = rows_sb[0:1, 2 * D:3 * D]
    bk2_row = rows_sb[0:1, 3 * D:4 * D]
    bqT_row = rows_sb[0:1, 4 * D:5 * D]
    bkT_row = rows_sb[0:1, 5 * D:6 * D]
    bv_col = consts.tile([128, H], BF, tag="bv")
    bo_col = consts.tile([128, H], F32, tag="bo")
    temp_col = consts.tile([128, H], F32, tag="temp")
    for dst, src in ((bv_col, d_bv), (bo_col, d_bo), (temp_col, d_temp)):
        nc.sync.dma_start(out=dst[:], in_=src[:, :])
    eyef = consts.tile([128, 128], F32, tag="eyef")
    eyeb = consts.tile([128, 128], BF, tag="eyeb")
    nc.sync.dma_start(out=eyef[:], in_=d_eyef[:, :])
    ones_col = consts.tile([128, 1], BF, tag="ones_c")
    nc.vector.memset(ones_col[:], 1.0)
    ones_row = consts.tile([1, 128], BF, tag="ones_r")
    nc.gpsimd.memset(ones_row[:], 1.0)
    epsln = consts.tile([128, 1], F32, tag="epsln")
    nc.vector.memset(epsln[:], EPS_LN)
    zcol = consts.tile([128, 1], F32, tag="zcol")
    nc.vector.memset(zcol[:], 0.0)

    rdm = pers.tile([128, NT, 2, 128], BF, tag="rdm")   # normalized R, d-major

    psSS = gacc.tile([128, 2, 256], F32, tag="psSS")
    psRX = gacc.tile([128, 2, 512], F32, tag="psRX")    # [M_RS | M_RR] blocks
    psSum = gacc.tile([128, 512], F32, tag="psSum")     # row 0: [s_S | s_R]

    # ================= phase 1: software-pipelined chunk stream ==========
    rawt = [None] * NCH
    sqt = [None] * NCH
    s1t = [None] * NCH
    s2t = [None] * NCH
    at_ = [None] * NCH
    bt_ = [None] * NCH
    ntt = [None] * NCH

    def p1_load(c):
        j0 = c * CHT
        raw = stage.tile([128, 2, CHT, 256], BF, tag="raw")
        nc.gpsimd.dma_start(out=raw[:, 0], in_=sv[:, j0:j0 + CHT, :])
        nc.gpsimd.dma_start(out=raw[:, 1], in_=rv[:, j0:j0 + CHT, :])
        rawt[c] = raw

    def p1_stats_a(c):
        raw = rawt[c]
        sq = scrp.tile([128, 2, CHT, 256], BF, tag="sq")
        nc.scalar.activation(out=sq[:], in_=raw[:], func=AF.Square,
                             bias=zcol[:, :], scale=1.0)
        s1 = stp.tile([128, 2, CHT], F32, tag="s1")
        nc.vector.tensor_reduce(out=s1[:], in_=raw[:], axis=AX.X, op=AL.add)
        sqt[c], s1t[c] = sq, s1

    def p1_stats_b(c):
        sq, s1 = sqt[c], s1t[c]
        s2 = stp.tile([128, 2, CHT], F32, tag="s2")
        nc.vector.tensor_reduce(out=s2[:], in_=sq[:], axis=AX.X, op=AL.add)
        p = stp.tile([128, 2, CHT], F32, tag="p")
        var = stp.tile([128, 2, CHT], F32, tag="var")
        sig = stp.tile([128, 2, CHT], F32, tag="sig")
        # var = s2 - s1^2/D ; sigma = sqrt(var/D + eps) via ACT scale
        nc.vector.scalar_tensor_tensor(out=p[:], in0=s1[:], scalar=-1.0 / D,
                                       op0=AL.mult, op1=AL.mult, in1=s1[:])
        nc.vector.scalar_tensor_tensor(out=var[:], in0=s2[:], scalar=1.0,
                                       op0=AL.bypass, op1=AL.add, in1=p[:])
        nc.scalar.activation(out=sig[:], in_=var[:], func=AF.Sqrt,
                             bias=epsln[:, :], scale=1.0 / D)
        a_sc = stp.tile([128, 2, CHT], F32, tag="a_sc")
        b_sc = stp.tile([128, 2, CHT], F32, tag="b_sc")
        nc.vector.reciprocal(out=a_sc[:], in_=sig[:])
        nc.vector.scalar_tensor_tensor(out=b_sc[:], in0=s1[:], scalar=-1.0 / D,
                                       op0=AL.mult, op1=AL.mult, in1=a_sc[:])
        at_[c], bt_[c] = a_sc, b_sc

    def p1_norms(c):
        raw, a_sc, b_sc = rawt[c], at_[c], bt_[c]
        nt = nrm.tile([128, 2, CHT, 256], BF, tag="nt")
        # balance: DVE gets 2 tiles, ACT gets 6
        for jj in range(CHT):
            if jj < 2:
                nc.vector.tensor_scalar(nt[:, 0, jj], raw[:, 0, jj],
                                        a_sc[:, 0, jj:jj + 1],
                                        b_sc[:, 0, jj:jj + 1], AL.mult, AL.add)
            else:
                nc.scalar.activation(out=nt[:, 0, jj], in_=raw[:, 0, jj],
                                     func=AF.Identity,
                                     bias=b_sc[:, 0, jj:jj + 1],
                                     scale=a_sc[:, 0, jj:jj + 1])
            nc.scalar.activation(out=nt[:, 1, jj], in_=raw[:, 1, jj],
                                 func=AF.Identity, bias=b_sc[:, 1, jj:jj + 1],
                                 scale=a_sc[:, 1, jj:jj + 1])
        ntt[c] = nt

    def p1_mms(c):
        j0 = c * CHT
        nt = ntt[c]
        for jj in range(CHT):
            j = j0 + jj
            st = (j == 0)
            sp = (j == NT - 1)
            for ech in range(2):
                nc.tensor.matmul(out=psSS[:, ech, :],
                                 lhsT=nt[:, 0, jj, ech * 128:(ech + 1) * 128],
                                 rhs=nt[:, 0, jj, :], start=st, stop=sp)
            for fch in range(2):
                nc.tensor.matmul(out=psRX[:, fch, :],
                                 lhsT=nt[:, 1, jj, fch * 128:(fch + 1) * 128],
                                 rhs=nt[:, :, jj, :], start=st, stop=sp)
            nc.tensor.matmul(out=psSum[0:1, :], lhsT=ones_col[:],
                             rhs=nt[:, :, jj, :], start=st, stop=sp)
        nc.sync.dma_start_transpose(rdm[:, j0:j0 + CHT, :, :], nt[:, 1])

    for i in range(NCH + 3):
        if i < NCH:
            p1_load(i)
        if 1 <= i <= NCH:
            p1_stats_a(i - 1)      # ACT square + DVE s1 (deps 1 iter old)
        if i >= 3:
            p1_norms(i - 3)        # DVE x2 + ACT x6 (deps 1 iter old)
            p1_mms(i - 3)
        if 2 <= i < NCH + 2:
            p1_stats_b(i - 2)      # DVE s2/fin + ACT sqrt tail

    # ================= phase 2: local reductions -> payload ================
    mSS = post.tile([128, 2, 256], BF, tag="mSS")
    mRS = post.tile([128, 2, 256], BF, tag="mRS")
    mRR = post.tile([128, 2, 256], BF, tag="mRR")
    nc.vector.tensor_scalar(mSS[:], psSS[:], 1.0, None, AL.mult)
    nc.scalar.activation(out=mRS[:], in_=psRX[:, :, 0:256], func=AF.Identity,
                         bias=zcol[:, :], scale=1.0)
    nc.scalar.activation(out=mRR[:], in_=psRX[:, :, 256:512], func=AF.Identity,
                         bias=zcol[:, :], scale=1.0)
    sums_sb = post.tile([1, 512], F32, tag="sums")
    nc.vector.tensor_scalar(sums_sb[:], psSum[0:1, :], 1.0, None, AL.mult)

    with tc.tile_pool(name="pps", bufs=2, space="PSUM") as pps:
        # s_S / s_R rows -> bf16 columns via PE transpose
        ps_sc = pps.tile([128, 2, 512], F32, tag="ps")
        for k in range(4):
            nc.tensor.transpose(ps_sc[:, 0, k:k + 1],
                                sums_sb[0:1, k * 128:(k + 1) * 128],
                                eyef[0:1, 0:1])
        scol = post.tile([128, 4], BF, tag="scol")   # sS e0,e1 | sR e0,e1
        nc.vector.tensor_scalar(scol[:], ps_sc[:, 0, 0:4], 1.0, None, AL.mult)

        # (Wq' s_S) and (Wk' s_R) as rows [1, 256]
        ps_r = pps.tile([128, 2, 512], F32, tag="ps")
        for ech in range(2):
            nc.tensor.matmul(out=ps_r[0:1, 0, 0:256], lhsT=scol[:, ech:ech + 1],
                             rhs=wqT[:, ech, :], start=(ech == 0),
                             stop=(ech == 1))
            nc.tensor.matmul(out=ps_r[0:1, 0, 256:512],
                             lhsT=scol[:, 2 + ech:3 + ech],
                             rhs=wkT[:, ech, :], start=(ech == 0),
                             stop=(ech == 1))
        prows = post.tile([1, 512], BF, tag="prows")  # wqss row | wksr row
        nc.vector.tensor_scalar(prows[:], ps_r[0:1, 0, :], 1.0, None, AL.mult)
        wqss_row = prows[0:1, 0:256]
        wksr_row = prows[0:1, 256:512]

        # V = M_SR Wk'^T  (lhsT = M_RS blocks)
        ps_v = pps.tile([128, 2, 512], F32, tag="ps")
        for ech in range(2):
            for fch in range(2):
                nc.tensor.matmul(out=ps_v[:, ech, 0:256],
                                 lhsT=mRS[:, fch, ech * 128:(ech + 1) * 128],
                                 rhs=wkT[:, fch, :], start=(fch == 0),
                                 stop=(fch == 1))
        v_sb = post.tile([128, 2, 256], BF, tag="v_sb")
        nc.scalar.activation(out=v_sb[:], in_=ps_v[:, :, 0:256],
                             func=AF.Identity, bias=zcol[:, :], scale=1.0)

        # Z_q = M_SS Wq'^T-ish, Z_k = M_RR Wk'^T (for norm diagonals)
        ps_z = pps.tile([128, 2, 512], F32, tag="ps")
        for ech in range(2):
            for fch in range(2):
                nc.tensor.matmul(out=ps_z[:, ech, 0:256],
                                 lhsT=mSS[:, fch, ech * 128:(ech + 1) * 128],
                                 rhs=wqT[:, fch, :], start=(fch == 0),
                                 stop=(fch == 1))
                nc.tensor.matmul(out=ps_z[:, ech, 256:512],
                                 lhsT=mRR[:, fch, ech * 128:(ech + 1) * 128],
                                 rhs=wkT[:, fch, :], start=(fch == 0),
                                 stop=(fch == 1))
        z_sb = post.tile([128, 2, 512], BF, tag="z_sb")
        nc.vector.tensor_scalar(z_sb[:], ps_z[:], 1.0, None, AL.mult)

        payload = post.tile([128, PAYW], F32, tag="payload")

        # G head blocks + bias outers
        ps_g = pps.tile([128, 2, 512], F32, tag="ps")
        for hh in range(2):
            g = ps_g[:, 0, hh * 128:(hh + 1) * 128]
            hs = slice(hh * 128, (hh + 1) * 128)
            for ech in range(2):
                nc.tensor.matmul(out=g, lhsT=wqT[:, ech, hs],
                                 rhs=v_sb[:, ech, hs], start=(ech == 0),
                                 stop=False)
            nc.tensor.matmul(out=g, lhsT=bq_row[:, hs], rhs=wksr_row[:, hs],
                             start=False, stop=False)
            nc.tensor.matmul(out=g, lhsT=wqss_row[:, hs], rhs=bk_row[:, hs],
                             start=False, stop=False)
            nc.tensor.matmul(out=g, lhsT=bq_row[:, hs], rhs=bkT_row[:, hs],
                             start=False, stop=True)
        nc.vector.tensor_scalar(payload[:, 0:256], ps_g[:, 0, 0:256], 1.0,
                                None, AL.mult)

        # Y_q / Y_k head blocks + bias outers; eye-dot -> dq, dk
        ps_y = pps.tile([128, 2, 512], F32, tag="ps")
        for hh in range(2):
            hs = slice(hh * 128, (hh + 1) * 128)
            yq = ps_y[:, hh, 0:128]
            yk = ps_y[:, hh, 128:256]
            for ech in range(2):
                nc.tensor.matmul(out=yq, lhsT=wqT[:, ech, hs],
                                 rhs=z_sb[:, ech, hs], start=(ech == 0),
                                 stop=False)
            nc.tensor.matmul(out=yq, lhsT=bq2_row[:, hs], rhs=wqss_row[:, hs],
                             start=False, stop=False)
            nc.tensor.matmul(out=yq, lhsT=bq_row[:, hs], rhs=bqT_row[:, hs],
                             start=False, stop=True)
            for ech in range(2):
                nc.tensor.matmul(
                    out=yk, lhsT=wkT[:, ech, hs],
                    rhs=z_sb[:, ech, 256 + hh * 128:256 + (hh + 1) * 128],
                    start=(ech == 0), stop=False)
            nc.tensor.matmul(out=yk, lhsT=bk2_row[:, hs], rhs=wksr_row[:, hs],
                             start=False, stop=False)
            nc.tensor.matmul(out=yk, lhsT=bk_row[:, hs], rhs=bkT_row[:, hs],
                             start=False, stop=True)
            dscr = small.tile([128, 128], F32, tag="dscr")
            nc.vector.scalar_tensor_tensor(
                out=dscr[:], in0=yq, scalar=1.0, op0=AL.mult, op1=AL.mult,
                in1=eyef[:], accum_out=payload[:, 256 + hh:257 + hh])
            nc.vector.scalar_tensor_tensor(
                out=dscr[:], in0=yk, scalar=1.0, op0=AL.mult, op1=AL.mult,
                in1=eyef[:], accum_out=payload[:, 258 + hh:259 + hh])

        # ---------------- collective ----------------
        cc_in = dram.tile([128, PAYW], F32)
        cc_out = dram.tile([128, PAYW], F32)
        nc.gpsimd.dma_start(out=cc_in[:, :], in_=payload[:])
        nc.gpsimd.collective_compute(
            "AllReduce", AL.add,
            replica_groups=[[0, 1], [2, 3], [4, 5], [6, 7]],
            ins=[cc_in.opt()], outs=[cc_out.opt()])
        # keep the PE HAM-warm through the collective window: harmless
        # 512-wide matmuls into the retired psSum bank (~14 us of busy)
        for _ in range(96):
            nc.tensor.matmul(out=psSum[:, :], lhsT=wqT[:, 0, 0:128],
                             rhs=wv_sb[:, 0:2, 0:256], start=True, stop=True)
        red = post.tile([128, PAYW], F32, tag="red")
        nc.sync.dma_start(out=red[:], in_=cc_out[:, :])

        # ---------------- phase 3: softmax + W_eff ----------------
        nrmc = small.tile([128, 4], F32, tag="nrmc")
        nc.scalar.activation(out=nrmc[:], in_=red[:, 256:260], func=AF.Sqrt,
                             bias=zcol[:, :], scale=1.0)
        nc.vector.tensor_scalar_max(nrmc[:], nrmc[:], EPS_NORM)
        nc.vector.reciprocal(out=nrmc[:], in_=nrmc[:])
        iq = small.tile([128, 2], F32, tag="iq")
        nc.vector.tensor_tensor(out=iq[:], in0=nrmc[:, 0:2], in1=temp_col[:],
                                op=AL.mult)

        # invk column -> broadcast tile via PE transpose + outer product
        ps_t = pps.tile([128, 2, 512], F32, tag="ps")
        ikrow = post.tile([1, 2, 128], BF, tag="ikrow")
        for hh in range(2):
            nc.tensor.transpose(ps_t[0:1, 0, hh * 128:(hh + 1) * 128],
                                nrmc[:, 2 + hh:3 + hh], eyef[:])
            nc.vector.tensor_scalar(
                ikrow[0:1, hh, :], ps_t[0:1, 0, hh * 128:(hh + 1) * 128],
                1.0, None, AL.mult)
        ps_ik = pps.tile([128, 2, 512], F32, tag="ps")
        for hh in range(2):
            nc.tensor.matmul(out=ps_ik[:, 0, hh * 128:(hh + 1) * 128],
                             lhsT=ones_row[:], rhs=ikrow[0:1, hh, :],
                             start=True, stop=True)
        ikb = post.tile([128, 2, 128], F32, tag="ikb")
        nc.vector.tensor_scalar(ikb[:], ps_ik[:, 0, 0:256], 1.0, None, AL.mult)

        # logits, softmax
        lg = post.tile([128, 2, 128], F32, tag="lg")
        for hh in range(2):
            nc.vector.tensor_scalar(lg[:, hh, :], red[:, hh * 128:(hh + 1) * 128],
                                    iq[:, hh:hh + 1], None, AL.mult)
        nc.vector.tensor_tensor(out=lg[:], in0=lg[:], in1=ikb[:], op=AL.mult)
        rmax = small.tile([128, 2], F32, tag="rmax")
        nc.vector.tensor_reduce(out=rmax[:], in_=lg[:], axis=AX.X, op=AL.max)
        nc.vector.tensor_scalar(rmax[:], rmax[:], -1.0, None, AL.mult)
        att = post.tile([128, 2, 128], F32, tag="att")
        for hh in range(2):
            nc.scalar.activation(out=att[:, hh, :], in_=lg[:, hh, :],
                                 func=AF.Exp, bias=rmax[:, hh:hh + 1],
                                 scale=1.0)
        rs = small.tile([128, 2], F32, tag="rs")
        nc.vector.tensor_reduce(out=rs[:], in_=att[:], axis=AX.X, op=AL.add)
        nc.vector.reciprocal(out=rs[:], in_=rs[:])
        attf = post.tile([128, 2, 128], F32, tag="attf")
        for hh in range(2):
            nc.vector.tensor_scalar(attf[:, hh, :], att[:, hh, :],
                                    rs[:, hh:hh + 1], None, AL.mult)

        # attn^T
        ps_at = pps.tile([128, 2, 512], F32, tag="ps")
        for hh in range(2):
            nc.tensor.transpose(ps_at[:, hh, 0:128], attf[:, hh, :], eyef[:])
        attT = post.tile([128, 2, 128], BF, tag="attT")
        nc.vector.tensor_scalar(attT[:], ps_at[:, :, 0:128], 1.0, None,
                                AL.mult)

        # A1_h = attn_h @ Wv'_h ; W_effT ; f2
        ps_a1 = pps.tile([128, 2, 512], F32, tag="ps")
        for hh in range(2):
            nc.tensor.matmul(out=ps_a1[:, hh, 0:256], lhsT=attT[:, hh, :],
                             rhs=wv_sb[:, hh, :], start=True, stop=True)
            nc.tensor.matmul(out=ps_a1[:, hh, 256:257], lhsT=attT[:, hh, :],
                             rhs=bv_col[:, hh:hh + 1], start=True, stop=True)
        a1 = post.tile([128, 2, 256], BF, tag="a1")
        nc.scalar.activation(out=a1[:], in_=ps_a1[:, :, 0:256],
                             func=AF.Identity, bias=zcol[:, :], scale=1.0)
        rc = small.tile([128, 2], BF, tag="rc")
        nc.vector.tensor_scalar(rc[:], ps_a1[:, :, 256], 1.0, None, AL.mult)

        ps_we = pps.tile([128, 2, 512], F32, tag="ps")
        for dch in range(2):
            for hh in range(2):
                nc.tensor.matmul(out=ps_we[:, dch, 0:256],
                                 lhsT=a1[:, hh, dch * 128:(dch + 1) * 128],
                                 rhs=woT[:, hh, :], start=(hh == 0),
                                 stop=(hh == 1))
        weT = post.tile([128, 2, 256], BF, tag="weT")
        nc.scalar.activation(out=weT[:], in_=ps_we[:, :, 0:256],
                             func=AF.Identity, bias=zcol[:, :], scale=1.0)

        # f2 = Wo rc + bo (column), -> row -> broadcast tile
        ps_f2 = pps.tile([128, 2, 512], F32, tag="ps")
        for och in range(2):
            for hh in range(2):
                nc.tensor.matmul(
                    out=ps_f2[:, 0, och:och + 1],
                    lhsT=woT[:, hh, och * 128:(och + 1) * 128],
                    rhs=rc[:, hh:hh + 1], start=(hh == 0), stop=(hh == 1))
        f2c = small.tile([128, 2], F32, tag="f2c")
        nc.vector.scalar_tensor_tensor(out=f2c[:], in0=ps_f2[:, 0, 0:2],
                                       scalar=1.0, op0=AL.bypass, op1=AL.add,
                                       in1=bo_col[:])
        ps_f2r = pps.tile([128, 2, 512], F32, tag="ps")
        f2row = post.tile([1, 2, 128], BF, tag="f2row")
        for och in range(2):
            nc.tensor.transpose(ps_f2r[0:1, 0, och * 128:(och + 1) * 128],
                                f2c[:, och:och + 1], eyef[:])
            nc.vector.tensor_scalar(
                f2row[0:1, och, :],
                ps_f2r[0:1, 0, och * 128:(och + 1) * 128], 1.0, None, AL.mult)
        ps_fb = pps.tile([128, 2, 512], F32, tag="ps")
        for och in range(2):
            nc.tensor.matmul(out=ps_fb[:, 0, och * 128:(och + 1) * 128],
                             lhsT=ones_row[:], rhs=f2row[0:1, och, :],
                             start=True, stop=True)
        f2b = post.tile([128, 256], F32, tag="f2b")
        nc.vector.tensor_scalar(f2b[:], ps_fb[:, 0, 0:256], 1.0, None, AL.mult)

    # ================= phase 4: output pass =================
    f2b2 = post.tile([128, 2, 256], F32, tag="f2b2")
    nc.vector.tensor_scalar(f2b2[:, 0, :], f2b[:], 1.0, None, AL.mult)
    nc.vector.tensor_scalar(f2b2[:, 1, :], f2b[:], 1.0, None, AL.mult)
    with tc.tile_pool(name="ops", bufs=4, space="PSUM") as ops:
        for g in range(NT // 4):
            osb = outp.tile([128, 4, 256], BF, tag="osb")
            for h2 in range(2):
                op_ps = ops.tile([128, 2, 256], F32, tag="op")
                for q2 in range(2):
                    j = g * 4 + h2 * 2 + q2
                    for dch in range(2):
                        nc.tensor.matmul(out=op_ps[:, q2, :],
                                         lhsT=rdm[:, j, dch, :],
                                         rhs=weT[:, dch, :], start=(dch == 0),
                                         stop=(dch == 1))
                nc.vector.scalar_tensor_tensor(
                    out=osb[:, h2 * 2:h2 * 2 + 2, :], in0=op_ps[:], scalar=1.0,
                    op0=AL.bypass, op1=AL.add, in1=f2b2[:])
            nc.sync.dma_start(out=outv[:, g * 4:(g + 1) * 4, :], in_=osb[:])


# ======================= host side =======================

def _prep_shared(inputs):
    f32 = np.float32
    Wq = np.asarray(inputs["Wq"], f32)
    bq = np.asarray(inputs["bq"], f32)
    Wkv = np.asarray(inputs["Wkv"], f32)
    bkv = np.asarray(inputs["bkv"], f32)
    Wo = np.asarray(inputs["Wo"], f32)
    bo = np.asarray(inputs["bo"], f32)
    lnS_w = np.asarray(inputs["lnS_w"], f32)
    lnS_b = np.asarray(inputs["lnS_b"], f32)
    lnR_w = np.asarray(inputs["lnR_w"], f32)
    lnR_b = np.asarray(inputs["lnR_b"], f32)
    temp = np.asarray(inputs["temperature"], f32).reshape(H)

    Wk, Wv = Wkv[:D], Wkv[D:]
    Wqp = Wq * lnS_w[None, :]
    Wkp = Wk * lnR_w[None, :]
    Wvp = Wv * lnR_w[None, :]
    bq2 = Wq @ lnS_b + bq
    bk2 = Wk @ lnR_b + bkv[:D]
    bv2 = Wv @ lnR_b + bkv[D:]

    def colh(v, dt=f32):
        return np.ascontiguousarray(v.reshape(H, 128).T).astype(dt)

    rows = np.concatenate([bq2, bk2, 2.0 * bq2, 2.0 * bk2,
                           float(T) * bq2, float(T) * bk2]).reshape(1, 6 * D)
    return {
        "wqT": np.ascontiguousarray(Wqp.T).astype(BF16),
        "wkT": np.ascontiguousarray(Wkp.T).astype(BF16),
        "wv": np.ascontiguousarray(Wvp).astype(BF16),
        "woT": np.ascontiguousarray(Wo.T).astype(BF16),
        "rows": rows.astype(BF16),
        "bv_col": colh(bv2, BF16),
        "bo_col": colh(bo),
        "temp_col": np.broadcast_to(temp[None, :], (128, H)).astype(f32).copy(),
        "eyef": np.eye(128, dtype=f32),
        "eyeb": np.eye(128, dtype=f32).astype(BF16),
    }


def _get_nc():
    if "nc" not in _nc_cache:
        _nc_cache["nc"] = _build_nc()
    return _nc_cache["nc"]


def run(inputs, trace=False):
    nc = _get_nc()
    shared = _prep_shared(inputs)
    iR = np.asarray(inputs["input_R"], np.float32)
    iS = np.asarray(inputs["input_S"], np.float32)
    in_maps = []
    for ci in range(N_CORES):
        b, half = ci // 2, ci % 2
        m = dict(shared)
        m["x_r"] = np.ascontiguousarray(iR[b, half * T:(half + 1) * T])
        m["x_s"] = np.ascontiguousarray(iS[b, half * T:(half + 1) * T])
        in_maps.append(m)
    res = run_bass_kernel_spmd(nc, in_maps, list(range(N_CORES)), trace=trace)
    out = np.zeros((B, N, D), np.float32)
    for ci in range(N_CORES):
        b, half = ci // 2, ci % 2
        out[b, half * T:(half + 1) * T] = np.asarray(
            res.results[ci]["out"]).astype(np.float32)
    return out, res


def kernel(**inputs):
    out, _ = run(inputs, trace=False)
    return out


# revision 16
# speedup vs baseline: 1.2655x; 1.1825x over previous
"""Channel-attention (XCA-style) Trainium2 kernel, 8-way SPMD — v2.

Shapes (hardcoded): B=4, N=16384, D=256, H=2 heads, c=128.
Sharding: core ci -> batch b=ci//2, token half ci%2 (T=8192 tokens/core).

Covariance formulation: per core, exactly LayerNorm the bf16 token tiles
(one fused scale+bias op per tile), accumulate three 256x256 token-
contracted Grams (M_SS, M_RS, M_RR) plus channel sums on the PE, then
  G    = Wq' M_SR Wk'^T + rank-1 bias outer-products   (head-diag blocks)
  dq/dk = diag(Wq' M_SS Wq'^T) + bias terms            (eye-dot on PE out)
One pair AllReduce of [128, 260] (G | dq | dk). Post-collective, softmax
gives attn; attn@v and the output projection collapse into a single
256x256 effective weight W_eff = Wo . blockdiag(attn_h) . Wv', applied to
the (transposed) normalized R in one matmul pass; per-token work in the
output phase is a single PSUM+bias-row evacuation. Output lands bf16 in
DRAM; the host upcasts to fp32.
"""
import sys, types

sys.path.insert(0, "/opt/trn_rl_repo")

try:
    import antenv
    if "antenv.axon_hooks" not in sys.modules:
        _hooks = types.ModuleType("antenv.axon_hooks")
        _hooks._hook = None
        _hooks.set_axon_ntff_profile_hook = lambda h: setattr(_hooks, "_hook", h)
        _hooks.get_axon_ntff_profile_hook = lambda: _hooks._hook
        sys.modules["antenv.axon_hooks"] = _hooks
        antenv.axon_hooks = _hooks
        from trn_agent_boot.trn_boot import _ntff_profile_via_ctypes
        _hooks.set_axon_ntff_profile_hook(
            _ntff_profile_via_ctypes("/opt/axon/libaxon_pjrt.so"))
except Exception:
    pass

import numpy as np
import ml_dtypes

import concourse.bass as bass
import concourse.bacc as bacc
import concourse.mybir as mybir
import concourse.tile as tile
from concourse.bass_utils import run_bass_kernel_spmd

BF16 = ml_dtypes.bfloat16
F32 = mybir.dt.float32
BF = mybir.dt.bfloat16
AL = mybir.AluOpType
AF = mybir.ActivationFunctionType
AX = mybir.AxisListType

B, N, D, H = 4, 16384, 256, 2
C = D // H
T = N // 2                 # tokens per core
NT = T // 128              # 64 token tiles / core
EPS_LN = 1e-5
EPS_NORM = 1e-12
N_CORES = 8
CHT = 4                    # token tiles per chunk
NCH = NT // CHT            # 8 chunks
PAYW = 260                 # collective payload width (G 256 | dq 2 | dk 2)

_nc_cache = {}


def _build_nc():
    nc = bacc.Bacc("TRN2", target_bir_lowering=False, debug=False,
                   num_devices=N_CORES)

    def ein(name, shape, dt=F32):
        return nc.dram_tensor(name, list(shape), dt, kind="ExternalInput")

    d_s = ein("x_s", [T, D])            # q source shard (input_S)
    d_r = ein("x_r", [T, D])            # kv source shard (input_R)
    d_wqT = ein("wqT", [D, D], BF)      # Wq'(=Wq.diag(lnS_w)) transposed [e,c]
    d_wkT = ein("wkT", [D, D], BF)
    d_wv = ein("wv", [D, D], BF)        # Wv' natural [c, e]
    d_woT = ein("woT", [D, D], BF)      # Wo transposed [c, o]
    d_rows = ein("rows", [1, 6 * D], BF)  # bq|bk|2bq|2bk|T*bq|T*bk rows
    d_bv = ein("bv_col", [128, H], BF)
    d_bo = ein("bo_col", [128, H])
    d_temp = ein("temp_col", [128, H])
    d_eyef = ein("eyef", [128, 128])
    d_eyeb = ein("eyeb", [128, 128], BF)
    d_out = nc.dram_tensor("out", [T, D], BF, kind="ExternalOutput")

    sv = d_s.rearrange("(j p) d -> p j d", p=128)
    rv = d_r.rearrange("(j p) d -> p j d", p=128)
    outv = d_out.rearrange("(j p) d -> p j d", p=128)

    with tile.TileContext(nc) as tc:
        import contextlib
        with contextlib.ExitStack() as ctx:
            _body(ctx, tc, nc, sv, rv, outv, d_wqT, d_wkT, d_wv, d_woT,
                  d_rows, d_bv, d_bo, d_temp, d_eyef, d_eyeb)
    nc.finalize()
    return nc


def _body(ctx, tc, nc, sv, rv, outv, d_wqT, d_wkT, d_wv, d_woT, d_rows,
          d_bv, d_bo, d_temp, d_eyef, d_eyeb):
    E = ctx.enter_context
    consts = E(tc.tile_pool(name="consts", bufs=1))
    stage = E(tc.tile_pool(name="stage", bufs=8))
    scrp = E(tc.tile_pool(name="scrp", bufs=3))
    nrm = E(tc.tile_pool(name="nrm", bufs=4))
    stp = E(tc.tile_pool(name="stp", bufs=4))
    pers = E(tc.tile_pool(name="pers", bufs=1))
    post = E(tc.tile_pool(name="post", bufs=1))
    small = E(tc.tile_pool(name="small", bufs=4))
    outp = E(tc.tile_pool(name="outp", bufs=2))
    dram = E(tc.tile_pool(name="dram", bufs=1, space="DRAM"))
    gacc = E(tc.tile_pool(name="gacc", bufs=1, space="PSUM"))

    # ---------------- constants ----------------
    wqT = consts.tile([128, 2, D], BF, tag="wqT")
    wkT = consts.tile([128, 2, D], BF, tag="wkT")
    wv_sb = consts.tile([128, 2, D], BF, tag="wv")
    woT = consts.tile([128, 2, D], BF, tag="woT")
    for dst, src in ((wqT, d_wqT), (wkT, d_wkT), (wv_sb, d_wv), (woT, d_woT)):
        nc.sync.dma_start(out=dst[:], in_=src.rearrange("(h p) o -> p h o", p=128))
    rows_sb = consts.tile([1, 6 * D], BF, tag="rows")
    nc.sync.dma_start(out=rows_sb[:], in_=d_rows[:, :])
    bq_row = rows_sb[0:1, 0 * D:1 * D]
    bk_row = rows_sb[0:1, 1 * D:2 * D]
    bq2_row = rows_sb[0:1, 2 * D:3 * D]
    bk2_row = rows_sb[0:1, 3 * D:4 * D]
    bqT_row = rows_sb[0:1, 4 * D:5 * D]
    bkT_row = rows_sb[0:1, 5 * D:6 * D]
    bv_col = consts.tile([128, H], BF, tag="bv")
    bo_col = consts.tile([128, H], F32, tag="bo")
    temp_col = consts.tile([128, H], F32, tag="temp")
    for dst, src in ((bv_col, d_bv), (bo_col, d_bo), (temp_col, d_temp)):
        nc.sync.dma_start(out=dst[:], in_=src[:, :])
    eyef = consts.tile([128, 128], F32, tag="eyef")
    eyeb = consts.tile([128, 128], BF, tag="eyeb")
    nc.sync.dma_start(out=eyef[:], in_=d_eyef[:, :])
    ones_col = consts.tile([128, 1], BF, tag="ones_c")
    nc.vector.memset(ones_col[:], 1.0)
    ones_row = consts.tile([1, 128], BF, tag="ones_r")
    nc.gpsimd.memset(ones_row[:], 1.0)
    epsln = consts.tile([128, 1], F32, tag="epsln")
    nc.vector.memset(epsln[:], EPS_LN)
    zcol = consts.tile([128, 1], F32, tag="zcol")
    nc.vector.memset(zcol[:], 0.0)

    rdm = pers.tile([128, NT, 2, 128], BF, tag="rdm")   # normalized R, d-major

    psSS = gacc.tile([128, 2, 256], F32, tag="psSS")
    psRX = gacc.tile([128, 2, 512], F32, tag="psRX")    # [M_RS | M_RR] blocks
    psSum = gacc.tile([128, 512], F32, tag="psSum")     # row 0: [s_S | s_R]

    # ================= phase 1: software-pipelined chunk stream ==========
    rawt = [None] * NCH
    sqt = [None] * NCH
    s1t = [None] * NCH
    s2t = [None] * NCH
    at_ = [None] * NCH
    bt_ = [None] * NCH
    ntt = [None] * NCH

    def p1_load(c):
        j0 = c * CHT
        raw = stage.tile([128, 2, CHT, 256], BF, tag="raw")
        nc.gpsimd.dma_start(out=raw[:, 0], in_=sv[:, j0:j0 + CHT, :])
        nc.gpsimd.dma_start(out=raw[:, 1], in_=rv[:, j0:j0 + CHT, :])
        rawt[c] = raw

    def p1_stats_a(c):
        raw = rawt[c]
        sq = scrp.tile([128, 2, CHT, 256], BF, tag="sq")
        nc.scalar.activation(out=sq[:], in_=raw[:], func=AF.Square,
                             bias=zcol[:, :], scale=1.0)
        s1 = stp.tile([128, 2, CHT], F32, tag="s1")
        nc.vector.tensor_reduce(out=s1[:], in_=raw[:], axis=AX.X, op=AL.add)
        sqt[c], s1t[c] = sq, s1

    def p1_stats_b(c):
        sq, s1 = sqt[c], s1t[c]
        s2 = stp.tile([128, 2, CHT], F32, tag="s2")
        nc.vector.tensor_reduce(out=s2[:], in_=sq[:], axis=AX.X, op=AL.add)
        p = stp.tile([128, 2, CHT], F32, tag="p")
        var = stp.tile([128, 2, CHT], F32, tag="var")
        sig = stp.tile([128, 2, CHT], F32, tag="sig")
        # var = s2 - s1^2/D ; sigma = sqrt(var/D + eps) via ACT scale
        nc.vector.scalar_tensor_tensor(out=p[:], in0=s1[:], scalar=-1.0 / D,
                                       op0=AL.mult, op1=AL.mult, in1=s1[:])
        nc.vector.scalar_tensor_tensor(out=var[:], in0=s2[:], scalar=1.0,
                                       op0=AL.bypass, op1=AL.add, in1=p[:])
        nc.scalar.activation(out=sig[:], in_=var[:], func=AF.Sqrt,
                             bias=epsln[:, :], scale=1.0 / D)
        a_sc = stp.tile([128, 2, CHT], F32, tag="a_sc")
        b_sc = stp.tile([128, 2, CHT], F32, tag="b_sc")
        nc.vector.reciprocal(out=a_sc[:], in_=sig[:])
        nc.vector.scalar_tensor_tensor(out=b_sc[:], in0=s1[:], scalar=-1.0 / D,
                                       op0=AL.mult, op1=AL.mult, in1=a_sc[:])
        at_[c], bt_[c] = a_sc, b_sc

    def p1_norms(c):
        raw, a_sc, b_sc = rawt[c], at_[c], bt_[c]
        nt = nrm.tile([128, 2, CHT, 256], BF, tag="nt")
        # balance: DVE gets 2 tiles, ACT gets 6
        for jj in range(CHT):
            if jj < 2:
                nc.vector.tensor_scalar(nt[:, 0, jj], raw[:, 0, jj],
                                        a_sc[:, 0, jj:jj + 1],
                                        b_sc[:, 0, jj:jj + 1], AL.mult, AL.add)
            else:
                nc.scalar.activation(out=nt[:, 0, jj], in_=raw[:, 0, jj],
                                     func=AF.Identity,
                                     bias=b_sc[:, 0, jj:jj + 1],
                                     scale=a_sc[:, 0, jj:jj + 1])
            nc.scalar.activation(out=nt[:, 1, jj], in_=raw[:, 1, jj],
                                 func=AF.Identity, bias=b_sc[:, 1, jj:jj + 1],
                                 scale=a_sc[:, 1, jj:jj + 1])
        ntt[c] = nt

    def p1_mms(c):
        j0 = c * CHT
        nt = ntt[c]
        for jj in range(CHT):
            j = j0 + jj
            st = (j == 0)
            sp = (j == NT - 1)
            for ech in range(2):
                nc.tensor.matmul(out=psSS[:, ech, :],
                                 lhsT=nt[:, 0, jj, ech * 128:(ech + 1) * 128],
                                 rhs=nt[:, 0, jj, :], start=st, stop=sp)
            for fch in range(2):
                nc.tensor.matmul(out=psRX[:, fch, :],
                                 lhsT=nt[:, 1, jj, fch * 128:(fch + 1) * 128],
                                 rhs=nt[:, :, jj, :], start=st, stop=sp)
            nc.tensor.matmul(out=psSum[0:1, :], lhsT=ones_col[:],
                             rhs=nt[:, :, jj, :], start=st, stop=sp)
        nc.sync.dma_start_transpose(rdm[:, j0:j0 + CHT, :, :], nt[:, 1])

    for i in range(NCH + 3):
        if i < NCH:
            p1_load(i)
        if 1 <= i <= NCH:
            p1_stats_a(i - 1)      # ACT square + DVE s1 (deps 1 iter old)
        if i >= 3:
            p1_norms(i - 3)        # DVE x2 + ACT x6 (deps 1 iter old)
            p1_mms(i - 3)
        if 2 <= i < NCH + 2:
            p1_stats_b(i - 2)      # DVE s2/fin + ACT sqrt tail

    # ================= phase 2: local reductions -> payload ================
    mSS = post.tile([128, 2, 256], BF, tag="mSS")
    mRS = post.tile([128, 2, 256], BF, tag="mRS")
    mRR = post.tile([128, 2, 256], BF, tag="mRR")
    nc.vector.tensor_scalar(mSS[:], psSS[:], 1.0, None, AL.mult)
    nc.scalar.activation(out=mRS[:], in_=psRX[:, :, 0:256], func=AF.Identity,
                         bias=zcol[:, :], scale=1.0)
    nc.scalar.activation(out=mRR[:], in_=psRX[:, :, 256:512], func=AF.Identity,
                         bias=zcol[:, :], scale=1.0)
    sums_sb = post.tile([1, 512], F32, tag="sums")
    nc.vector.tensor_scalar(sums_sb[:], psSum[0:1, :], 1.0, None, AL.mult)

    with tc.tile_pool(name="pps", bufs=2, space="PSUM") as pps:
        # s_S / s_R rows -> bf16 columns via PE transpose
        ps_sc = pps.tile([128, 2, 512], F32, tag="ps")
        for k in range(4):
            nc.tensor.transpose(ps_sc[:, 0, k:k + 1],
                                sums_sb[0:1, k * 128:(k + 1) * 128],
                                eyef[0:1, 0:1])
        scol = post.tile([128, 4], BF, tag="scol")   # sS e0,e1 | sR e0,e1
        nc.vector.tensor_scalar(scol[:], ps_sc[:, 0, 0:4], 1.0, None, AL.mult)

        # (Wq' s_S) and (Wk' s_R) as rows [1, 256]
        ps_r = pps.tile([128, 2, 512], F32, tag="ps")
        for ech in range(2):
            nc.tensor.matmul(out=ps_r[0:1, 0, 0:256], lhsT=scol[:, ech:ech + 1],
                             rhs=wqT[:, ech, :], start=(ech == 0),
                             stop=(ech == 1))
            nc.tensor.matmul(out=ps_r[0:1, 0, 256:512],
                             lhsT=scol[:, 2 + ech:3 + ech],
                             rhs=wkT[:, ech, :], start=(ech == 0),
                             stop=(ech == 1))
        prows = post.tile([1, 512], BF, tag="prows")  # wqss row | wksr row
        nc.vector.tensor_scalar(prows[:], ps_r[0:1, 0, :], 1.0, None, AL.mult)
        wqss_row = prows[0:1, 0:256]
        wksr_row = prows[0:1, 256:512]

        # V = M_SR Wk'^T  (lhsT = M_RS blocks)
        ps_v = pps.tile([128, 2, 512], F32, tag="ps")
        for ech in range(2):
            for fch in range(2):
                nc.tensor.matmul(out=ps_v[:, ech, 0:256],
                                 lhsT=mRS[:, fch, ech * 128:(ech + 1) * 128],
                                 rhs=wkT[:, fch, :], start=(fch == 0),
                                 stop=(fch == 1))
        v_sb = post.tile([128, 2, 256], BF, tag="v_sb")
        nc.scalar.activation(out=v_sb[:], in_=ps_v[:, :, 0:256],
                             func=AF.Identity, bias=zcol[:, :], scale=1.0)

        # Z_q = M_SS Wq'^T-ish, Z_k = M_RR Wk'^T (for norm diagonals)
        ps_z = pps.tile([128, 2, 512], F32, tag="ps")
        for ech in range(2):
            for fch in range(2):
                nc.tensor.matmul(out=ps_z[:, ech, 0:256],
                                 lhsT=mSS[:, fch, ech * 128:(ech + 1) * 128],
                                 rhs=wqT[:, fch, :], start=(fch == 0),
                                 stop=(fch == 1))
                nc.tensor.matmul(out=ps_z[:, ech, 256:512],
                                 lhsT=mRR[:, fch, ech * 128:(ech + 1) * 128],
                                 rhs=wkT[:, fch, :], start=(fch == 0),
                                 stop=(fch == 1))
        z_sb = post.tile([128, 2, 512], BF, tag="z_sb")
        nc.vector.tensor_scalar(z_sb[:], ps_z[:], 1.0, None, AL.mult)

        payload = post.tile([128, PAYW], F32, tag="payload")

        # G head blocks + bias outers
        ps_g = pps.tile([128, 2, 512], F32, tag="ps")
        for hh in range(2):
            g = ps_g[:, 0, hh * 128:(hh + 1) * 128]
            hs = slice(hh * 128, (hh + 1) * 128)
            for ech in range(2):
                nc.tensor.matmul(out=g, lhsT=wqT[:, ech, hs],
                                 rhs=v_sb[:, ech, hs], start=(ech == 0),
                                 stop=False)
            nc.tensor.matmul(out=g, lhsT=bq_row[:, hs], rhs=wksr_row[:, hs],
                             start=False, stop=False)
            nc.tensor.matmul(out=g, lhsT=wqss_row[:, hs], rhs=bk_row[:, hs],
                             start=False, stop=False)
            nc.tensor.matmul(out=g, lhsT=bq_row[:, hs], rhs=bkT_row[:, hs],
                             start=False, stop=True)
        nc.vector.tensor_scalar(payload[:, 0:256], ps_g[:, 0, 0:256], 1.0,
                                None, AL.mult)

        # Y_q / Y_k head blocks + bias outers; eye-dot -> dq, dk
        ps_y = pps.tile([128, 2, 512], F32, tag="ps")
        for hh in range(2):
            hs = slice(hh * 128, (hh + 1) * 128)
            yq = ps_y[:, hh, 0:128]
            yk = ps_y[:, hh, 128:256]
            for ech in range(2):
                nc.tensor.matmul(out=yq, lhsT=wqT[:, ech, hs],
                                 rhs=z_sb[:, ech, hs], start=(ech == 0),
                                 stop=False)
            nc.tensor.matmul(out=yq, lhsT=bq2_row[:, hs], rhs=wqss_row[:, hs],
                             start=False, stop=False)
            nc.tensor.matmul(out=yq, lhsT=bq_row[:, hs], rhs=bqT_row[:, hs],
                             start=False, stop=True)
            for ech in range(2):
                nc.tensor.matmul(
                    out=yk, lhsT=wkT[:, ech, hs],
                    rhs=z_sb[:, ech, 256 + hh * 128:256 + (hh + 1) * 128],
                    start=(ech == 0), stop=False)
            nc.tensor.matmul(out=yk, lhsT=bk2_row[:, hs], rhs=wksr_row[:, hs],
                             start=False, stop=False)
            nc.tensor.matmul(out=yk, lhsT=bk_row[:, hs], rhs=bkT_row[:, hs],
                             start=False, stop=True)
            dscr = small.tile([128, 128], F32, tag="dscr")
            nc.vector.scalar_tensor_tensor(
                out=dscr[:], in0=yq, scalar=1.0, op0=AL.mult, op1=AL.mult,
                in1=eyef[:], accum_out=payload[:, 256 + hh:257 + hh])
            nc.vector.scalar_tensor_tensor(
                out=dscr[:], in0=yk, scalar=1.0, op0=AL.mult, op1=AL.mult,
                in1=eyef[:], accum_out=payload[:, 258 + hh:259 + hh])

        # ---------------- collective ----------------
        cc_in = dram.tile([128, PAYW], F32)
        cc_out = dram.tile([128, PAYW], F32)
        nc.gpsimd.dma_start(out=cc_in[:, :], in_=payload[:])
        nc.gpsimd.collective_compute(
            "AllReduce", AL.add,
            replica_groups=[[0, 1], [2, 3], [4, 5], [6, 7]],
            ins=[cc_in.opt()], outs=[cc_out.opt()])
        # keep the PE HAM-warm through the collective window: harmless
        # 512-wide matmuls into the retired psSum bank (~14 us of busy)
        for _ in range(96):
            nc.tensor.matmul(out=psSum[:, :], lhsT=wqT[:, 0, 0:128],
                             rhs=wv_sb[:, 0:2, 0:256], start=True, stop=True)
        red = post.tile([128, PAYW], F32, tag="red")
        nc.sync.dma_start(out=red[:], in_=cc_out[:, :])

        # ---------------- phase 3: softmax + W_eff ----------------
        nrmc = small.tile([128, 4], F32, tag="nrmc")
        nc.scalar.activation(out=nrmc[:], in_=red[:, 256:260], func=AF.Sqrt,
                             bias=zcol[:, :], scale=1.0)
        nc.vector.tensor_scalar_max(nrmc[:], nrmc[:], EPS_NORM)
        nc.vector.reciprocal(out=nrmc[:], in_=nrmc[:])
        iq = small.tile([128, 2], F32, tag="iq")
        nc.vector.tensor_tensor(out=iq[:], in0=nrmc[:, 0:2], in1=temp_col[:],
                                op=AL.mult)

        # invk column -> broadcast tile via PE transpose + outer product
        ps_t = pps.tile([128, 2, 512], F32, tag="ps")
        ikrow = post.tile([1, 2, 128], BF, tag="ikrow")
        for hh in range(2):
            nc.tensor.transpose(ps_t[0:1, 0, hh * 128:(hh + 1) * 128],
                                nrmc[:, 2 + hh:3 + hh], eyef[:])
            nc.vector.tensor_scalar(
                ikrow[0:1, hh, :], ps_t[0:1, 0, hh * 128:(hh + 1) * 128],
                1.0, None, AL.mult)
        ps_ik = pps.tile([128, 2, 512], F32, tag="ps")
        for hh in range(2):
            nc.tensor.matmul(out=ps_ik[:, 0, hh * 128:(hh + 1) * 128],
                             lhsT=ones_row[:], rhs=ikrow[0:1, hh, :],
                             start=True, stop=True)
        ikb = post.tile([128, 2, 128], F32, tag="ikb")
        nc.vector.tensor_scalar(ikb[:], ps_ik[:, 0, 0:256], 1.0, None, AL.mult)

        # logits, softmax
        lg = post.tile([128, 2, 128], F32, tag="lg")
        for hh in range(2):
            nc.vector.tensor_scalar(lg[:, hh, :], red[:, hh * 128:(hh + 1) * 128],
                                    iq[:, hh:hh + 1], None, AL.mult)
        nc.vector.tensor_tensor(out=lg[:], in0=lg[:], in1=ikb[:], op=AL.mult)
        rmax = small.tile([128, 2], F32, tag="rmax")
        nc.vector.tensor_reduce(out=rmax[:], in_=lg[:], axis=AX.X, op=AL.max)
        nc.vector.tensor_scalar(rmax[:], rmax[:], -1.0, None, AL.mult)
        att = post.tile([128, 2, 128], F32, tag="att")
        for hh in range(2):
            nc.scalar.activation(out=att[:, hh, :], in_=lg[:, hh, :],
                                 func=AF.Exp, bias=rmax[:, hh:hh + 1],
                                 scale=1.0)
        rs = small.tile([128, 2], F32, tag="rs")
        nc.vector.tensor_reduce(out=rs[:], in_=att[:], axis=AX.X, op=AL.add)
        nc.vector.reciprocal(out=rs[:], in_=rs[:])
        attf = post.tile([128, 2, 128], F32, tag="attf")
        for hh in range(2):
            nc.vector.tensor_scalar(attf[:, hh, :], att[:, hh, :],
                                    rs[:, hh:hh + 1], None, AL.mult)

        # attn^T
        ps_at = pps.tile([128, 2, 512], F32, tag="ps")
        for hh in range(2):
            nc.tensor.transpose(ps_at[:, hh, 0:128], attf[:, hh, :], eyef[:])
        attT = post.tile([128, 2, 128], BF, tag="attT")
        nc.vector.tensor_scalar(attT[:], ps_at[:, :, 0:128], 1.0, None,
                                AL.mult)

        # A1_h = attn_h @ Wv'_h ; W_effT ; f2
        ps_a1 = pps.tile([128, 2, 512], F32, tag="ps")
        for hh in range(2):
            nc.tensor.matmul(out=ps_a1[:, hh, 0:256], lhsT=attT[:, hh, :],
                             rhs=wv_sb[:, hh, :], start=True, stop=True)
            nc.tensor.matmul(out=ps_a1[:, hh, 256:257], lhsT=attT[:, hh, :],
                             rhs=bv_col[:, hh:hh + 1], start=True, stop=True)
        a1 = post.tile([128, 2, 256], BF, tag="a1")
        nc.scalar.activation(out=a1[:], in_=ps_a1[:, :, 0:256],
                             func=AF.Identity, bias=zcol[:, :], scale=1.0)
        rc = small.tile([128, 2], BF, tag="rc")
        nc.vector.tensor_scalar(rc[:], ps_a1[:, :, 256], 1.0, None, AL.mult)

        ps_we = pps.tile([128, 2, 512], F32, tag="ps")
        for dch in range(2):
            for hh in range(2):
                nc.tensor.matmul(out=ps_we[:, dch, 0:256],
                                 lhsT=a1[:, hh, dch * 128:(dch + 1) * 128],
                                 rhs=woT[:, hh, :], start=(hh == 0),
                                 stop=(hh == 1))
        weT = post.tile([128, 2, 256], BF, tag="weT")
        nc.scalar.activation(out=weT[:], in_=ps_we[:, :, 0:256],
                             func=AF.Identity, bias=zcol[:, :], scale=1.0)

        # f2 = Wo rc + bo (column), -> row -> broadcast tile
        ps_f2 = pps.tile([128, 2, 512], F32, tag="ps")
        for och in range(2):
            for hh in range(2):
                nc.tensor.matmul(
                    out=ps_f2[:, 0, och:och + 1],
                    lhsT=woT[:, hh, och * 128:(och + 1) * 128],
                    rhs=rc[:, hh:hh + 1], start=(hh == 0), stop=(hh == 1))
        f2c = small.tile([128, 2], F32, tag="f2c")
        nc.vector.scalar_tensor_tensor(out=f2c[:], in0=ps_f2[:, 0, 0:2],
                                       scalar=1.0, op0=AL.bypass, op1=AL.add,
                                       in1=bo_col[:])
        ps_f2r = pps.tile([128, 2, 512], F32, tag="ps")
        f2row = post.tile([1, 2, 128], BF, tag="f2row")
        for och in range(2):
            nc.tensor.transpose(ps_f2r[0:1, 0, och * 128:(och + 1) * 128],
                                f2c[:, och:och + 1], eyef[:])
            nc.vector.tensor_scalar(
                f2row[0:1, och, :],
                ps_f2r[0:1, 0, och * 128:(och + 1) * 128], 1.0, None, AL.mult)
        ps_fb = pps.tile([128, 2, 512], F32, tag="ps")
        for och in range(2):
            nc.tensor.matmul(out=ps_fb[:, 0, och * 128:(och + 1) * 128],
                             lhsT=ones_row[:], rhs=f2row[0:1, och, :],
                             start=True, stop=True)
        f2b = post.tile([128, 256], F32, tag="f2b")
        nc.vector.tensor_scalar(f2b[:], ps_fb[:, 0, 0:256], 1.0, None, AL.mult)

    # ================= phase 4: output pass =================
    f2b2 = post.tile([128, 2, 256], F32, tag="f2b2")
    nc.vector.tensor_scalar(f2b2[:, 0, :], f2b[:], 1.0, None, AL.mult)
    nc.vector.tensor_scalar(f2b2[:, 1, :], f2b[:], 1.0, None, AL.mult)
    with tc.tile_pool(name="ops", bufs=4, space="PSUM") as ops:
        for g in range(NT // 4):
            osb = outp.tile([128, 4, 256], BF, tag="osb")
            for h2 in range(2):
                op_ps = ops.tile([128, 2, 256], F32, tag="op")
                for q2 in range(2):
                    j = g * 4 + h2 * 2 + q2
                    for dch in range(2):
                        nc.tensor.matmul(out=op_ps[:, q2, :],
                                         lhsT=rdm[:, j, dch, :],
                                         rhs=weT[:, dch, :], start=(dch == 0),
                                         stop=(dch == 1))
                nc.vector.scalar_tensor_tensor(
                    out=osb[:, h2 * 2:h2 * 2 + 2, :], in0=op_ps[:], scalar=1.0,
                    op0=AL.bypass, op1=AL.add, in1=f2b2[:])
            nc.sync.dma_start(out=outv[:, g * 4:(g + 1) * 4, :], in_=osb[:])


# ======================= host side =======================

def _prep_shared(inputs):
    f32 = np.float32
    Wq = np.asarray(inputs["Wq"], f32)
    bq = np.asarray(inputs["bq"], f32)
    Wkv = np.asarray(inputs["Wkv"], f32)
    bkv = np.asarray(inputs["bkv"], f32)
    Wo = np.asarray(inputs["Wo"], f32)
    bo = np.asarray(inputs["bo"], f32)
    lnS_w = np.asarray(inputs["lnS_w"], f32)
    lnS_b = np.asarray(inputs["lnS_b"], f32)
    lnR_w = np.asarray(inputs["lnR_w"], f32)
    lnR_b = np.asarray(inputs["lnR_b"], f32)
    temp = np.asarray(inputs["temperature"], f32).reshape(H)

    Wk, Wv = Wkv[:D], Wkv[D:]
    Wqp = Wq * lnS_w[None, :]
    Wkp = Wk * lnR_w[None, :]
    Wvp = Wv * lnR_w[None, :]
    bq2 = Wq @ lnS_b + bq
    bk2 = Wk @ lnR_b + bkv[:D]
    bv2 = Wv @ lnR_b + bkv[D:]

    def colh(v, dt=f32):
        return np.ascontiguousarray(v.reshape(H, 128).T).astype(dt)

    rows = np.concatenate([bq2, bk2, 2.0 * bq2, 2.0 * bk2,
                           float(T) * bq2, float(T) * bk2]).reshape(1, 6 * D)
    return {
        "wqT": np.ascontiguousarray(Wqp.T).astype(BF16),
        "wkT": np.ascontiguousarray(Wkp.T).astype(BF16),
        "wv": np.ascontiguousarray(Wvp).astype(BF16),
        "woT": np.ascontiguousarray(Wo.T).astype(BF16),
        "rows": rows.astype(BF16),
        "bv_col": colh(bv2, BF16),
        "bo_col": colh(bo),
        "temp_col": np.broadcast_to(temp[None, :], (128, H)).astype(f32).copy(),
        "eyef": np.eye(128, dtype=f32),
        "eyeb": np.eye(128, dtype=f32).astype(BF16),
    }


def _get_nc():
    if "nc" not in _nc_cache:
        _nc_cache["nc"] = _build_nc()
    return _nc_cache["nc"]


def run(inputs, trace=False):
    nc = _get_nc()
    shared = _prep_shared(inputs)
    iR = np.asarray(inputs["input_R"], np.float32)
    iS = np.asarray(inputs["input_S"], np.float32)
    in_maps = []
    for ci in range(N_CORES):
        b, half = ci // 2, ci % 2
        m = dict(shared)
        m["x_r"] = np.ascontiguousarray(iR[b, half * T:(half + 1) * T])
        m["x_s"] = np.ascontiguousarray(iS[b, half * T:(half + 1) * T])
        in_maps.append(m)
    res = run_bass_kernel_spmd(nc, in_maps, list(range(N_CORES)), trace=trace)
    out = np.zeros((B, N, D), np.float32)
    for ci in range(N_CORES):
        b, half = ci // 2, ci % 2
        out[b, half * T:(half + 1) * T] = np.asarray(
            res.results[ci]["out"]).astype(np.float32)
    return out, res


def kernel(**inputs):
    out, _ = run(inputs, trace=False)
    return out
